# revision 12
# baseline (speedup 1.0000x reference)
"""Sliding-window causal self-attention (GQA + RoPE + RMS-norm + value-embedding
gate) for Trainium2, sharded over 8 NeuronCores.

Sharding: sequence-parallel. (batch=2) x (4 sequence chunks of 1024) = 8 shards.
Each core computes attention for its own 1024 query rows. Window size = 1024 and
chunk size = 1024, so each core only needs K/V for its own chunk plus the
previous 1024 positions (halo). K/V (+rope/rms/gate) are recomputed locally for
the halo instead of communicated -> zero collectives. Chunk-0 shards get a
zero-padded halo; padded keys produce k=0 => exp(score)=1, which is excluded
from the softmax denominator by loading a per-row validity column into the
augmented-V "ones" column (padded v rows are 0 so the numerator is untouched).

v2 (fused pipeline):
- Single software-pipelined loop: kv-production K(rt), q-projection Q(qt),
  attention A(h,qt) and out-projection D(qt) interleave so the PE never idles
  between phases.
- All [128,128] transposes go through the DMA XBAR (dma_start_transpose)
  instead of PE matmul transposes + DVE psum->sbuf copies.
- Activation engine runs only {Exp, Square, Copy, Identity} (one table set,
  zero mid-kernel table reloads): the gate sigmoid is computed as
  1/(1+exp(-x)) and the RMS rsqrt as a DVE tensor_scalar pow(x, -0.5).
- RMS mean-square uses Act Square with fused accum_out.
- Rope multiplies/adds and edge masks on DVE (bf16 packed 4x mode); psum->sbuf
  copies and the v-gate fused multiply-add on the Pool engine.
- Scores are computed pre-transposed (s[k,q]) so exp's bf16 output is directly
  the lhsT of the PV matmul; V is augmented with the validity column so the PV
  matmul emits y[q,0:128] and the softmax denominator Z=y[q,128] in one
  accumulation group; normalization is a single DVE divide.
- Input DMAs are streamed in sequence slices so compute starts ~4us in.
"""

import math
import sys

import numpy as np

sys.path.insert(0, "/opt/trn_rl_repo")

import ml_dtypes

import concourse.bass as bass
import concourse.bacc as bacc
import concourse.tile as tile
from concourse import mybir
from concourse import bass_utils

BF16 = ml_dtypes.bfloat16
F32 = np.float32

B, T, C = 2, 4096, 1024
H, HKV, D = 8, 2, 128
REP = H // HKV
WIN = 1024
RCHUNK = 1024          # own rows per core
E = 2048               # ext rows (halo + own)
NRT = E // 128         # 16 ext row tiles
NQT = RCHUNK // 128    # 8 q tiles
NKC = 9                # k chunks per q tile
NKA = 5                # first exp group (chunks 0..4)
NKB = NKC - NKA        # second exp group (chunks 5..8)
NCT = C // 128         # 8 contraction tiles
NSL = 8                # xT/cs/ve DMA stream slices over E
SLE = E // NSL         # rows per stream slice
EPS = float(np.finfo(np.float32).eps)
SCALE = 1.0 / math.sqrt(D)

dt = mybir.dt
AF = mybir.ActivationFunctionType
ALU = mybir.AluOpType
AX = mybir.AxisListType


def _bcast(ap, n, axis_pos=1):
    """Insert a 0-stride dim of size n into an AP at free-axis position."""
    new_ap = list(ap.ap)
    new_ap.insert(axis_pos, [0, n])
    return bass.AP(tensor=ap.tensor, offset=ap.offset, ap=new_ap)


def _halfswap(ap, nh):
    """View [128, nh, 128] with the two 64-wide halves of the last dim
    swapped: out[p, h, 0:64] = in[p, h, 64:128] and vice versa."""
    base = list(ap.ap)
    return bass.AP(tensor=ap.tensor, offset=ap.offset + 64,
                   ap=[base[0], base[1], [-64, 2], [1, 64]])


def build_nc():
    nc = bacc.Bacc("TRN2", target_bir_lowering=False, debug=False)

    xT_d = nc.dram_tensor("xT", [C, E], dt.bfloat16, kind="ExternalInput").ap()
    wq_d = nc.dram_tensor("wq", [C, C], dt.bfloat16, kind="ExternalInput").ap()
    wkv_d = nc.dram_tensor("wkv", [C, 512], dt.bfloat16, kind="ExternalInput").ap()
    wo_d = nc.dram_tensor("wo", [C, C], dt.bfloat16, kind="ExternalInput").ap()
    wg_d = nc.dram_tensor("wg", [32, HKV], dt.bfloat16, kind="ExternalInput").ap()
    ve_d = nc.dram_tensor("ve2", [E, HKV * D], dt.bfloat16, kind="ExternalInput").ap()
    cs_d = nc.dram_tensor("cs", [E, 256], dt.bfloat16, kind="ExternalInput").ap()
    tri_d = nc.dram_tensor("tri", [128, 2 * 128], dt.bfloat16, kind="ExternalInput").ap()
    vm_d = nc.dram_tensor("vmask", [128, NRT * HKV], dt.bfloat16,
                          kind="ExternalInput").ap()
    out_d = nc.dram_tensor("out", [RCHUNK, C], dt.bfloat16, kind="ExternalOutput").ap()

    with tile.TileContext(nc) as tc:
        _body(tc, xT_d, wq_d, wkv_d, wo_d, wg_d, ve_d, cs_d, tri_d, vm_d, out_d)
    nc.compile()
    return nc


def _body(tc, xT_d, wq_d, wkv_d, wo_d, wg_d, ve_d, cs_d, tri_d, vm_d, out_d):
    nc = tc.nc
    from contextlib import ExitStack

    with ExitStack() as ctx:
        const = ctx.enter_context(tc.tile_pool(name="const", bufs=1))
        persist = ctx.enter_context(tc.tile_pool(name="persist", bufs=1))
        work = ctx.enter_context(tc.tile_pool(name="work", bufs=2))
        pwork = ctx.enter_context(tc.tile_pool(name="pwork", bufs=3))
        mmps = ctx.enter_context(tc.tile_pool(name="mmps", bufs=3, space="PSUM"))
        saps = ctx.enter_context(tc.tile_pool(name="saps", bufs=1, space="PSUM"))
        sbps = ctx.enter_context(tc.tile_pool(name="sbps", bufs=1, space="PSUM"))
        yps = ctx.enter_context(tc.tile_pool(name="yps", bufs=2, space="PSUM"))

        # ---- persistent SBUF ----
        wkv_sb = const.tile([128, NCT, 512], dt.bfloat16)
        wg_sb = const.tile([32, HKV], dt.bfloat16)
        xT_sb = const.tile([128, NCT, E], dt.bfloat16)
        cs_sb = const.tile([128, NRT, 256], dt.bfloat16)
        ve_sb = const.tile([128, NRT, HKV * D], dt.bfloat16)
        wq_sb = const.tile([128, NCT, C], dt.bfloat16)
        tri_sb = const.tile([128, 2, 128], dt.bfloat16)
        vm_sb = const.tile([128, NRT * HKV], dt.bfloat16)
        wo_sb = const.tile([128, NCT, C], dt.bfloat16)

        kT_sb = persist.tile([128, HKV, NRT, 128], dt.bfloat16)   # [d, kvh, g, k]
        v_sb = persist.tile([128, NRT, HKV, 129], dt.bfloat16)    # [k, g, kvh, d|m]
        gate_sb = persist.tile([128, NRT * HKV], dt.float32)      # [row, (g,kvh)]
        eg_sb = persist.tile([128, NRT * HKV], dt.float32)
        msk_sb = persist.tile([128, NRT, HKV], dt.float32)
        msq_sb = persist.tile([128, NQT, H], dt.float32)
        eps_sb = persist.tile([128, 1], dt.float32)
        lns_sb = persist.tile([128, 1], dt.float32)
        nc.vector.memset(eps_sb, EPS)
        nc.vector.memset(lns_sb, math.log(SCALE))

        # DMA views (partition-tiled DRAM)
        xTv = xT_d.rearrange("(a p) n -> p a n", p=128)
        csv = cs_d.rearrange("(a p) n -> p a n", p=128)
        vev = ve_d.rearrange("(a p) n -> p a n", p=128)

        # ---- streamed input DMAs, earliest-needed first ----
        nc.sync.dma_start(out=wkv_sb, in_=wkv_d.rearrange("(a p) n -> p a n", p=128))
        nc.sync.dma_start(out=wg_sb, in_=wg_d)

        def load_slice(sl):
            r = bass.ts(sl, SLE)
            nc.sync.dma_start(out=xT_sb[:, :, r], in_=xTv[:, :, r])
            rt0 = sl * (NRT // NSL)
            nc.sync.dma_start(out=cs_sb[:, rt0:rt0 + NRT // NSL, :],
                              in_=csv[:, rt0:rt0 + NRT // NSL, :])
            nc.sync.dma_start(out=ve_sb[:, rt0:rt0 + NRT // NSL, :],
                              in_=vev[:, rt0:rt0 + NRT // NSL, :])

        load_slice(0)
        load_slice(1)
        nc.sync.dma_start(out=tri_sb, in_=tri_d.rearrange("p (a n) -> p a n", a=2))
        nc.sync.dma_start(out=vm_sb, in_=vm_d)
        load_slice(2)
        load_slice(3)
        load_slice(4)
        nc.sync.dma_start(out=wq_sb, in_=wq_d.rearrange("(a p) n -> p a n", p=128))
        for sl in range(5, NSL):
            load_slice(sl)
        nc.sync.dma_start(out=wo_sb, in_=wo_d.rearrange("(a p) n -> p a n", p=128))

        # validity column of augmented V (kills the softmax pad correction)
        nc.vector.tensor_copy(
            out=v_sb[:, :, :, 128:129].rearrange("p a b n -> p (a b n)"),
            in_=vm_sb)

        # ---- gates: 2*sigmoid(u) = 2/(1+exp(-u)); the 2x is folded into ve2
        g_psum = mmps.tile([128, 512], dt.float32, tag="mm")
        for rt in range(NRT):
            nc.tensor.matmul(g_psum[:, bass.ts(rt, HKV)],
                             lhsT=xT_sb[0:32, 0, bass.ts(rt, 128)],
                             rhs=wg_sb, start=True, stop=True)
        nc.scalar.activation(out=eg_sb, in_=g_psum[:, 0:NRT * HKV],
                             func=AF.Exp, scale=-1.0)
        nc.vector.tensor_scalar_add(out=eg_sb, in0=eg_sb, scalar1=1.0)
        nc.vector.reciprocal(out=gate_sb, in_=eg_sb)

        # ---------------- pipeline stages ----------------
        def K(rt):
            """kv projection + gate-mix + rope + rms + normalize + kT."""
            rs = bass.ts(rt, 128)
            kv = mmps.tile([128, 512], dt.float32, tag="mm")
            for ct in range(NCT):
                nc.tensor.matmul(kv, lhsT=xT_sb[:, ct, rs], rhs=wkv_sb[:, ct, :],
                                 start=(ct == 0), stop=(ct == NCT - 1))
            # v = v_raw + gate*ve2
            for kvh in range(HKV):
                nc.vector.scalar_tensor_tensor(
                    out=v_sb[:, rt, kvh, 0:128],
                    in0=ve_sb[:, rt, bass.ts(kvh, 128)],
                    scalar=gate_sb[:, rt * HKV + kvh:rt * HKV + kvh + 1],
                    in1=kv[:, 256 + kvh * 128:256 + (kvh + 1) * 128],
                    op0=ALU.mult, op1=ALU.add)
            kraw = work.tile([128, HKV * D], dt.bfloat16, tag="kraw")
            nc.scalar.copy(out=kraw, in_=kv[:, 0:256])
            # rope (DVE, bf16 packed)
            k3 = kraw.rearrange("p (a n) -> p a n", a=HKV)
            kr = work.tile([128, HKV, D], dt.bfloat16, tag="krot")
            kc_ = work.tile([128, HKV, D], dt.bfloat16, tag="kc")
            ks_ = work.tile([128, HKV, D], dt.bfloat16, tag="ks")
            ccb = _bcast(cs_sb[:, rt, 0:128], HKV)
            ssb = _bcast(cs_sb[:, rt, 128:256], HKV)
            nc.vector.tensor_mul(kc_, k3, ccb)
            nc.vector.tensor_mul(ks_, k3, ssb)
            nc.vector.tensor_add(kr, kc_, _halfswap(ks_, HKV))
            # rms stats from the ROPED values (exact for any cos/sin)
            sqs = work.tile([128, HKV, D], dt.bfloat16, tag="sqs")
            nc.scalar.activation(out=sqs.rearrange("p a n -> p (a n)"),
                                 in_=kr.rearrange("p a n -> p (a n)"),
                                 func=AF.Square)
            nc.vector.tensor_reduce(out=msk_sb[:, rt, :], in_=sqs,
                                    axis=AX.X, op=ALU.add)
            # c = rsqrt(ms/D + eps) = Exp(-0.5 * Ln(ms/D + eps)); Ln and Exp
            # live in one act table set so the Act engine never reloads.
            nc.scalar.activation(out=msk_sb[:, rt, :], in_=msk_sb[:, rt, :],
                                 func=AF.Ln, scale=1.0 / D, bias=eps_sb)
            nc.scalar.activation(out=msk_sb[:, rt, :], in_=msk_sb[:, rt, :],
                                 func=AF.Exp, scale=-0.5)
            for kvh in range(HKV):
                nc.vector.tensor_scalar_mul(
                    kr[:, kvh, :], kr[:, kvh, :], msk_sb[:, rt, kvh:kvh + 1])
                nc.sync.dma_start_transpose(out=kT_sb[:, kvh, rt, :],
                                            in_=kr[:, kvh, :])

        def Q(qt):
            """q projection + rope + rms + normalize(+scale) + qT."""
            rt = NQT + qt  # ext row tile of own rows
            rs = bass.ts(rt, 128)
            qraw = work.tile([128, C], dt.bfloat16, tag="qraw")
            for half in range(2):
                qp = mmps.tile([128, 512], dt.float32, tag="mm")
                for ct in range(NCT):
                    nc.tensor.matmul(qp, lhsT=xT_sb[:, ct, rs],
                                     rhs=wq_sb[:, ct, bass.ts(half, 512)],
                                     start=(ct == 0), stop=(ct == NCT - 1))
                nc.scalar.copy(out=qraw[:, bass.ts(half, 512)], in_=qp)
            q3 = qraw.rearrange("p (a n) -> p a n", a=H)
            qr = work.tile([128, H, D], dt.bfloat16, tag="qrot")
            qc_ = work.tile([128, H, D], dt.bfloat16, tag="qc")
            qs_ = work.tile([128, H, D], dt.bfloat16, tag="qs")
            ccb = _bcast(cs_sb[:, rt, 0:128], H)
            ssb = _bcast(cs_sb[:, rt, 128:256], H)
            nc.vector.tensor_mul(qc_, q3, ccb)
            nc.vector.tensor_mul(qs_, q3, ssb)
            nc.vector.tensor_add(qr, qc_, _halfswap(qs_, H))
            sqs = work.tile([128, H, D], dt.bfloat16, tag="qsqs")
            nc.scalar.activation(out=sqs.rearrange("p a n -> p (a n)"),
                                 in_=qr.rearrange("p a n -> p (a n)"),
                                 func=AF.Square)
            nc.vector.tensor_reduce(out=msq_sb[:, qt, :], in_=sqs,
                                    axis=AX.X, op=ALU.add)
            # c = rsqrt(ms/D + eps) * SCALE = Exp(-0.5*Ln(ms/D+eps) + ln SCALE)
            nc.scalar.activation(out=msq_sb[:, qt, :], in_=msq_sb[:, qt, :],
                                 func=AF.Ln, scale=1.0 / D, bias=eps_sb)
            nc.scalar.activation(out=msq_sb[:, qt, :], in_=msq_sb[:, qt, :],
                                 func=AF.Exp, scale=-0.5, bias=lns_sb)
            qT = pwork.tile([128, H, 128], dt.bfloat16, tag="qT")
            for h in range(H):
                nc.vector.tensor_scalar_mul(
                    qr[:, h, :], qr[:, h, :], msq_sb[:, qt, h:h + 1])
                nc.sync.dma_start_transpose(out=qT[:, h, :], in_=qr[:, h, :])
            return qT

        def A(h, qt, qT, yN):
            """one head x q-tile of windowed attention."""
            kvh = h // REP
            p = pwork.tile([128, NKC, 128], dt.bfloat16, tag="p")
            sa = saps.tile([128, NKA, 128], dt.float32, tag="sa")
            for kc in range(NKA):
                nc.tensor.matmul(sa[:, kc, :], lhsT=kT_sb[:, kvh, qt + kc, :],
                                 rhs=qT[:, h, :], start=True, stop=True)
            nc.scalar.activation(
                out=p[:, 0:NKA, :].rearrange("p a n -> p (a n)"),
                in_=sa.rearrange("p a n -> p (a n)"), func=AF.Exp)
            sb = sbps.tile([128, NKB, 128], dt.float32, tag="sb")
            for kc in range(NKB):
                nc.tensor.matmul(sb[:, kc, :],
                                 lhsT=kT_sb[:, kvh, qt + NKA + kc, :],
                                 rhs=qT[:, h, :], start=True, stop=True)
            nc.scalar.activation(
                out=p[:, NKA:NKC, :].rearrange("p a n -> p (a n)"),
                in_=sb.rearrange("p a n -> p (a n)"), func=AF.Exp)
            # triangular edge masks (left chunk kc=0, diag chunk kc=8)
            nc.gpsimd.tensor_mul(p[:, 0, :], p[:, 0, :], tri_sb[:, 0, :])
            nc.gpsimd.tensor_mul(p[:, NKC - 1, :], p[:, NKC - 1, :],
                                 tri_sb[:, 1, :])
            y = yps.tile([128, 129], dt.float32, tag="y")
            for kc in range(NKC):
                nc.tensor.matmul(y, lhsT=p[:, kc, :],
                                 rhs=v_sb[:, qt + kc, kvh, :],
                                 start=(kc == 0), stop=(kc == NKC - 1))
            # yN = y * (1/Z)  (Z = y[:,128], already pad-free via vmask col)
            z = work.tile([128, 1], dt.float32, tag="z")
            nc.vector.reciprocal(out=z, in_=y[:, 128:129])
            nc.vector.tensor_scalar_mul(yN[:, h, :], y[:, 0:128], z)

        def Dout(qt, yN):
            """transpose yN + output projection + store."""
            yT = pwork.tile([128, H, 128], dt.bfloat16, tag="yT")
            for h in range(H):
                nc.sync.dma_start_transpose(out=yT[:, h, :], in_=yN[:, h, :])
            for half in range(2):
                o = mmps.tile([128, 512], dt.float32, tag="mm")
                for h in range(H):
                    nc.tensor.matmul(o, lhsT=yT[:, h, :],
                                     rhs=wo_sb[:, h, bass.ts(half, 512)],
                                     start=(h == 0), stop=(h == H - 1))
                osb = work.tile([128, 512], dt.bfloat16, tag="osb")
                nc.vector.tensor_copy(out=osb, in_=o)
                nc.sync.dma_start(
                    out=out_d[bass.ts(qt, 128), bass.ts(half, 512)], in_=osb)

        # ---------------- emission schedule ----------------
        for rt in range(NQT + 1):      # K(0..8)
            K(rt)
        qT_cur = Q(0)
        yN_cur = pwork.tile([128, H, 128], dt.bfloat16, tag="yN")
        for qt in range(NQT):
            if qt + NQT + 1 < NRT:
                K(qt + NQT + 1)
            for h in range(H):
                A(h, qt, qT_cur, yN_cur)
                # prefetch next q tile early, between attention iterations
                if h == 1 and qt + 1 < NQT:
                    qT_nxt = Q(qt + 1)
            Dout(qt, yN_cur)
            if qt + 1 < NQT:
                qT_cur = qT_nxt
                yN_cur = pwork.tile([128, H, 128], dt.bfloat16, tag="yN")


# ---------------------------------------------------------------------------
# host side
# ---------------------------------------------------------------------------

def make_in_maps(x, ve, cos, sin, Wq, Wk, Wv, Wproj, Wg):
    """Build the 8 per-core input dicts (numpy, host-side prep)."""
    x = np.asarray(x, F32)
    ve = np.asarray(ve, F32)
    cos = np.asarray(cos, F32).reshape(T, 64)
    sin = np.asarray(sin, F32).reshape(T, 64)
    Wq = np.asarray(Wq, F32)
    Wk = np.asarray(Wk, F32)
    Wv = np.asarray(Wv, F32)
    Wproj = np.asarray(Wproj, F32)
    Wg = np.asarray(Wg, F32)

    wq = Wq.astype(BF16)
    wkv = np.concatenate([Wk, Wv], axis=1).astype(BF16)
    wo = Wproj.astype(BF16)
    wg = Wg.astype(BF16)

    # triangular masks in [k, q] layout
    kk = np.arange(128)[:, None]
    qq = np.arange(128)[None, :]
    tri = np.zeros((128, 2, 128), F32)
    tri[:, 0, :] = np.where(kk < qq, 0.0, 1.0)   # LEFT chunk (kc=0), mult mask
    tri[:, 1, :] = np.where(kk > qq, 0.0, 1.0)   # DIAG chunk (kc=8), mult mask
    tri = tri.reshape(128, 256).astype(BF16)

    in_maps = []
    for c in range(8):
        b, ck = divmod(c, 4)
        t0 = ck * RCHUNK
        es = t0 - WIN  # ext start (may be negative for chunk 0)
        pad = max(0, -es)

        def ext(a, fill_shape):
            out = np.zeros((E,) + fill_shape, F32)
            out[pad:] = a[es + pad: t0 + RCHUNK]
            return out

        x_e = ext(x[b], (C,))
        ve_e = ext(ve[b], (HKV * D,))
        cos_e = ext(cos, (64,))
        sin_e = ext(sin, (64,))

        # validity of each ext row, laid out [128, (rt, kvh)] to drop into
        # the augmented-V column: Z then automatically excludes padded rows.
        extpos = (np.arange(NRT)[None, :] * 128 + np.arange(128)[:, None])
        valid = (extpos >= pad).astype(F32)                  # [128, NRT]
        vmask = np.repeat(valid, HKV, axis=1).astype(BF16)   # [128, NRT*HKV]

        in_maps.append({
            "xT": np.ascontiguousarray(x_e.T).astype(BF16),
            "wq": wq, "wkv": wkv, "wo": wo, "wg": wg,
            "ve2": (2.0 * ve_e).astype(BF16),
            "cs": np.concatenate([cos_e, cos_e, -sin_e, sin_e],
                                 axis=1).astype(BF16),
            "tri": tri, "vmask": vmask,
        })
    return in_maps


_NC_CACHE = None


def kernel(x, ve, cos, sin, Wq, Wk, Wv, Wproj, Wg, window_size):
    assert int(window_size) == WIN
    global _NC_CACHE
    if _NC_CACHE is None:
        _NC_CACHE = build_nc()
    nc = _NC_CACHE
    in_maps = make_in_maps(x, ve, cos, sin, Wq, Wk, Wv, Wproj, Wg)
    res = bass_utils.run_bass_kernel_spmd(nc, in_maps, core_ids=list(range(8)))
    out = np.zeros((B, T, C), F32)
    for c in range(8):
        b, ck = divmod(c, 4)
        out[b, ck * RCHUNK:(ck + 1) * RCHUNK] = res.results[c]["out"].astype(F32)
    return out


# revision 16
# speedup vs baseline: 1.2126x; 1.2126x over previous
"""Sliding-window causal self-attention (GQA + RoPE + RMS-norm + value-embedding
gate) for Trainium2, sharded over 8 NeuronCores.

Sharding: sequence-parallel. (batch=2) x (4 sequence chunks of 1024) = 8 shards.
Each core computes attention for its own 1024 query rows. Window size = 1024 and
chunk size = 1024, so each core only needs K/V for its own chunk plus the
previous 1024 positions (halo). K/V (+rope/rms/gate) are recomputed locally for
the halo instead of communicated -> zero collectives. Chunk-0 shards get a
zero-padded halo; padded keys produce k=0 => exp(score)=1, which is excluded
from the softmax denominator by loading a per-row validity column into the
augmented-V "ones" column (padded v rows are 0 so the numerator is untouched).

v2 (fused pipeline):
- Single software-pipelined loop: kv-production K(rt), q-projection Q(qt),
  attention A(h,qt) and out-projection D(qt) interleave so the PE never idles
  between phases.
- All [128,128] transposes go through the DMA XBAR (dma_start_transpose)
  instead of PE matmul transposes + DVE psum->sbuf copies.
- Activation engine runs only {Exp, Square, Copy, Identity} (one table set,
  zero mid-kernel table reloads): the gate sigmoid is computed as
  1/(1+exp(-x)) and the RMS rsqrt as a DVE tensor_scalar pow(x, -0.5).
- RMS mean-square uses Act Square with fused accum_out.
- Rope multiplies/adds and edge masks on DVE (bf16 packed 4x mode); psum->sbuf
  copies and the v-gate fused multiply-add on the Pool engine.
- Scores are computed pre-transposed (s[k,q]) so exp's bf16 output is directly
  the lhsT of the PV matmul; V is augmented with the validity column so the PV
  matmul emits y[q,0:128] and the softmax denominator Z=y[q,128] in one
  accumulation group; normalization is a single DVE divide.
- Input DMAs are streamed in sequence slices so compute starts ~4us in.
"""

import math
import sys

import numpy as np

sys.path.insert(0, "/opt/trn_rl_repo")

import ml_dtypes

import concourse.bass as bass
import concourse.bacc as bacc
import concourse.tile as tile
from concourse import mybir
from concourse import bass_utils

BF16 = ml_dtypes.bfloat16
F32 = np.float32

B, T, C = 2, 4096, 1024
H, HKV, D = 8, 2, 128
REP = H // HKV
WIN = 1024
RCHUNK = 1024          # own rows per core
E = 2048               # ext rows (halo + own)
NRT = E // 128         # 16 ext row tiles
NQT = RCHUNK // 128    # 8 q tiles
NKC = 9                # k chunks per q tile
NKA = 5                # first exp group (chunks 0..4)
NKB = NKC - NKA        # second exp group (chunks 5..8)
NCT = C // 128         # 8 contraction tiles
NSL = 8                # xT/cs/ve DMA stream slices over E
SLE = E // NSL         # rows per stream slice
EPS = float(np.finfo(np.float32).eps)
SCALE = 1.0 / math.sqrt(D)

dt = mybir.dt
AF = mybir.ActivationFunctionType
ALU = mybir.AluOpType
AX = mybir.AxisListType


def _bcast(ap, n, axis_pos=1):
    """Insert a 0-stride dim of size n into an AP at free-axis position."""
    new_ap = list(ap.ap)
    new_ap.insert(axis_pos, [0, n])
    return bass.AP(tensor=ap.tensor, offset=ap.offset, ap=new_ap)


def _halfswap(ap, nh):
    """View [128, nh, 128] with the two 64-wide halves of the last dim
    swapped: out[p, h, 0:64] = in[p, h, 64:128] and vice versa."""
    base = list(ap.ap)
    return bass.AP(tensor=ap.tensor, offset=ap.offset + 64,
                   ap=[base[0], base[1], [-64, 2], [1, 64]])


def build_nc():
    nc = bacc.Bacc("TRN2", target_bir_lowering=False, debug=False)

    xT_d = nc.dram_tensor("xT", [C, E], dt.bfloat16, kind="ExternalInput").ap()
    wq_d = nc.dram_tensor("wq", [C, C], dt.bfloat16, kind="ExternalInput").ap()
    wkv_d = nc.dram_tensor("wkv", [C, 512], dt.bfloat16, kind="ExternalInput").ap()
    wo_d = nc.dram_tensor("wo", [C, C], dt.bfloat16, kind="ExternalInput").ap()
    wg_d = nc.dram_tensor("wg", [32, HKV], dt.bfloat16, kind="ExternalInput").ap()
    ve_d = nc.dram_tensor("ve2", [E, HKV * D], dt.bfloat16, kind="ExternalInput").ap()
    cs_d = nc.dram_tensor("cs", [E, 256], dt.bfloat16, kind="ExternalInput").ap()
    tri_d = nc.dram_tensor("tri", [128, 2 * 128], dt.bfloat16, kind="ExternalInput").ap()
    vm_d = nc.dram_tensor("vmask", [128, NRT * HKV], dt.bfloat16,
                          kind="ExternalInput").ap()
    out_d = nc.dram_tensor("out", [RCHUNK, C], dt.bfloat16, kind="ExternalOutput").ap()

    with tile.TileContext(nc) as tc:
        _body(tc, xT_d, wq_d, wkv_d, wo_d, wg_d, ve_d, cs_d, tri_d, vm_d, out_d)
    nc.compile()
    return nc


def _body(tc, xT_d, wq_d, wkv_d, wo_d, wg_d, ve_d, cs_d, tri_d, vm_d, out_d):
    nc = tc.nc
    from contextlib import ExitStack

    with ExitStack() as ctx:
        const = ctx.enter_context(tc.tile_pool(name="const", bufs=1))
        persist = ctx.enter_context(tc.tile_pool(name="persist", bufs=1))
        work = ctx.enter_context(tc.tile_pool(name="work", bufs=2))
        pwork = ctx.enter_context(tc.tile_pool(name="pwork", bufs=3))
        mmps = ctx.enter_context(tc.tile_pool(name="mmps", bufs=3, space="PSUM"))
        saps = ctx.enter_context(tc.tile_pool(name="saps", bufs=1, space="PSUM"))
        sbps = ctx.enter_context(tc.tile_pool(name="sbps", bufs=1, space="PSUM"))
        yps = ctx.enter_context(tc.tile_pool(name="yps", bufs=2, space="PSUM"))

        # ---- persistent SBUF ----
        wkv_sb = const.tile([128, NCT, 512], dt.bfloat16)
        wg_sb = const.tile([32, HKV], dt.bfloat16)
        xT_sb = const.tile([128, NCT, E], dt.bfloat16)
        cs_sb = const.tile([128, NRT, 256], dt.bfloat16)
        ve_sb = const.tile([128, NRT, HKV * D], dt.bfloat16)
        wq_sb = const.tile([128, NCT, C], dt.bfloat16)
        tri_sb = const.tile([128, 2, 128], dt.bfloat16)
        vm_sb = const.tile([128, NRT * HKV], dt.bfloat16)
        wo_sb = const.tile([128, NCT, C], dt.bfloat16)

        kT_sb = persist.tile([128, HKV, NRT, 128], dt.bfloat16)   # [d, kvh, g, k]
        v_sb = persist.tile([128, NRT, HKV, 129], dt.bfloat16)    # [k, g, kvh, d|m]
        gate_sb = persist.tile([128, NRT * HKV], dt.float32)      # [row, (g,kvh)]
        eg_sb = persist.tile([128, NRT * HKV], dt.float32)
        msk_sb = persist.tile([128, NRT, HKV], dt.float32)
        msq_sb = persist.tile([128, NQT, H], dt.float32)
        eps_sb = persist.tile([128, 1], dt.float32)
        lns_sb = persist.tile([128, 1], dt.float32)
        nc.vector.memset(eps_sb, EPS)
        nc.vector.memset(lns_sb, math.log(SCALE))

        # DMA views (partition-tiled DRAM)
        xTv = xT_d.rearrange("(a p) n -> p a n", p=128)
        csv = cs_d.rearrange("(a p) n -> p a n", p=128)
        vev = ve_d.rearrange("(a p) n -> p a n", p=128)

        # ---- streamed input DMAs, earliest-needed first ----
        nc.sync.dma_start(out=wkv_sb, in_=wkv_d.rearrange("(a p) n -> p a n", p=128))
        nc.sync.dma_start(out=wg_sb, in_=wg_d)

        def load_slice(sl):
            r = bass.ts(sl, SLE)
            nc.sync.dma_start(out=xT_sb[:, :, r], in_=xTv[:, :, r])
            rt0 = sl * (NRT // NSL)
            nc.sync.dma_start(out=cs_sb[:, rt0:rt0 + NRT // NSL, :],
                              in_=csv[:, rt0:rt0 + NRT // NSL, :])
            nc.sync.dma_start(out=ve_sb[:, rt0:rt0 + NRT // NSL, :],
                              in_=vev[:, rt0:rt0 + NRT // NSL, :])

        load_slice(0)
        load_slice(1)
        nc.sync.dma_start(out=tri_sb, in_=tri_d.rearrange("p (a n) -> p a n", a=2))
        nc.sync.dma_start(out=vm_sb, in_=vm_d)
        load_slice(2)
        load_slice(3)
        load_slice(4)
        nc.sync.dma_start(out=wq_sb, in_=wq_d.rearrange("(a p) n -> p a n", p=128))
        for sl in range(5, NSL):
            load_slice(sl)
        nc.sync.dma_start(out=wo_sb, in_=wo_d.rearrange("(a p) n -> p a n", p=128))

        # Pin the act-function table to the one set containing every func we
        # use (Exp, Ln, Square, Copy, Identity) so the fixpoint table-load
        # pass never inserts another (otherwise Ln<->Exp thrashes 1.3us/load).
        from concourse.hw_specs import get_activation_tables
        _need = {AF.Exp, AF.Ln, AF.Square, AF.Copy, AF.Identity}
        _set_id = next(i for i, (_, s) in
                       enumerate(get_activation_tables(nc.m.arch).items())
                       if _need <= s)
        _lat = mybir.InstLoadActFuncSet(
            name=nc.get_next_instruction_name(), ins=[], outs=[],
            act_func_set_id=_set_id)
        _lat.engine = mybir.EngineType.Activation
        nc.scalar.add_instruction(_lat)

        # validity column of augmented V (kills the softmax pad correction)
        nc.vector.tensor_copy(
            out=v_sb[:, :, :, 128:129].rearrange("p a b n -> p (a b n)"),
            in_=vm_sb)

        # ---- gates: 2*sigmoid(u) = 2/(1+exp(-u)); the 2x is folded into ve2
        g_psum = mmps.tile([128, 512], dt.float32, tag="mm")
        for rt in range(NRT):
            nc.tensor.matmul(g_psum[:, bass.ts(rt, HKV)],
                             lhsT=xT_sb[0:32, 0, bass.ts(rt, 128)],
                             rhs=wg_sb, start=True, stop=True)
        nc.scalar.activation(out=eg_sb, in_=g_psum[:, 0:NRT * HKV],
                             func=AF.Exp, scale=-1.0)
        nc.vector.tensor_scalar_add(out=eg_sb, in0=eg_sb, scalar1=1.0)
        nc.vector.reciprocal(out=gate_sb, in_=eg_sb)

        # ---------------- pipeline stages ----------------
        def K(rt):
            """kv projection + gate-mix + rope + rms + normalize + kT."""
            rs = bass.ts(rt, 128)
            kv = mmps.tile([128, 512], dt.float32, tag="mm")
            for ct in range(NCT):
                nc.tensor.matmul(kv, lhsT=xT_sb[:, ct, rs], rhs=wkv_sb[:, ct, :],
                                 start=(ct == 0), stop=(ct == NCT - 1))
            # v = v_raw + gate*ve2
            for kvh in range(HKV):
                nc.vector.scalar_tensor_tensor(
                    out=v_sb[:, rt, kvh, 0:128],
                    in0=ve_sb[:, rt, bass.ts(kvh, 128)],
                    scalar=gate_sb[:, rt * HKV + kvh:rt * HKV + kvh + 1],
                    in1=kv[:, 256 + kvh * 128:256 + (kvh + 1) * 128],
                    op0=ALU.mult, op1=ALU.add)
            kraw = work.tile([128, HKV * D], dt.bfloat16, tag="kraw")
            nc.scalar.copy(out=kraw, in_=kv[:, 0:256])
            # rope (DVE, bf16 packed)
            k3 = kraw.rearrange("p (a n) -> p a n", a=HKV)
            kr = work.tile([128, HKV, D], dt.bfloat16, tag="krot")
            kc_ = work.tile([128, HKV, D], dt.bfloat16, tag="kc")
            ks_ = work.tile([128, HKV, D], dt.bfloat16, tag="ks")
            ccb = _bcast(cs_sb[:, rt, 0:128], HKV)
            ssb = _bcast(cs_sb[:, rt, 128:256], HKV)
            nc.vector.tensor_mul(kc_, k3, ccb)
            nc.vector.tensor_mul(ks_, k3, ssb)
            nc.vector.tensor_add(kr, kc_, _halfswap(ks_, HKV))
            # rms stats from the ROPED values (exact for any cos/sin)
            sqs = work.tile([128, HKV, D], dt.bfloat16, tag="sqs")
            nc.scalar.activation(out=sqs.rearrange("p a n -> p (a n)"),
                                 in_=kr.rearrange("p a n -> p (a n)"),
                                 func=AF.Square)
            nc.vector.tensor_reduce(out=msk_sb[:, rt, :], in_=sqs,
                                    axis=AX.X, op=ALU.add)
            # c = rsqrt(ms/D + eps) = Exp(-0.5 * Ln(ms/D + eps)); Ln and Exp
            # live in one act table set so the Act engine never reloads.
            nc.scalar.activation(out=msk_sb[:, rt, :], in_=msk_sb[:, rt, :],
                                 func=AF.Ln, scale=1.0 / D, bias=eps_sb)
            nc.scalar.activation(out=msk_sb[:, rt, :], in_=msk_sb[:, rt, :],
                                 func=AF.Exp, scale=-0.5)
            for kvh in range(HKV):
                nc.vector.tensor_scalar_mul(
                    kr[:, kvh, :], kr[:, kvh, :], msk_sb[:, rt, kvh:kvh + 1])
            # one batched XBAR transpose for both kv heads: out[j,kvh,i]=in[i,kvh,j]
            nc.sync.dma_start_transpose(out=kT_sb[:, :, rt, :],
                                        in_=kr.rearrange("p a n -> p (a n)"))

        def Q(qt):
            """q projection + rope + rms + normalize(+scale) + qT."""
            rt = NQT + qt  # ext row tile of own rows
            rs = bass.ts(rt, 128)
            qraw = work.tile([128, C], dt.bfloat16, tag="qraw")
            for half in range(2):
                qp = mmps.tile([128, 512], dt.float32, tag="mm")
                for ct in range(NCT):
                    nc.tensor.matmul(qp, lhsT=xT_sb[:, ct, rs],
                                     rhs=wq_sb[:, ct, bass.ts(half, 512)],
                                     start=(ct == 0), stop=(ct == NCT - 1))
                nc.scalar.copy(out=qraw[:, bass.ts(half, 512)], in_=qp)
            q3 = qraw.rearrange("p (a n) -> p a n", a=H)
            qr = work.tile([128, H, D], dt.bfloat16, tag="qrot")
            qc_ = work.tile([128, H, D], dt.bfloat16, tag="qc")
            qs_ = work.tile([128, H, D], dt.bfloat16, tag="qs")
            ccb = _bcast(cs_sb[:, rt, 0:128], H)
            ssb = _bcast(cs_sb[:, rt, 128:256], H)
            nc.vector.tensor_mul(qc_, q3, ccb)
            nc.vector.tensor_mul(qs_, q3, ssb)
            nc.vector.tensor_add(qr, qc_, _halfswap(qs_, H))
            sqs = work.tile([128, H, D], dt.bfloat16, tag="qsqs")
            nc.scalar.activation(out=sqs.rearrange("p a n -> p (a n)"),
                                 in_=qr.rearrange("p a n -> p (a n)"),
                                 func=AF.Square)
            nc.vector.tensor_reduce(out=msq_sb[:, qt, :], in_=sqs,
                                    axis=AX.X, op=ALU.add)
            # c = rsqrt(ms/D + eps) * SCALE = Exp(-0.5*Ln(ms/D+eps) + ln SCALE)
            nc.scalar.activation(out=msq_sb[:, qt, :], in_=msq_sb[:, qt, :],
                                 func=AF.Ln, scale=1.0 / D, bias=eps_sb)
            nc.scalar.activation(out=msq_sb[:, qt, :], in_=msq_sb[:, qt, :],
                                 func=AF.Exp, scale=-0.5, bias=lns_sb)
            qT = pwork.tile([128, H, 128], dt.bfloat16, tag="qT")
            for h in range(H):
                nc.vector.tensor_scalar_mul(
                    qr[:, h, :], qr[:, h, :], msq_sb[:, qt, h:h + 1])
            nc.sync.dma_start_transpose(out=qT,
                                        in_=qr.rearrange("p a n -> p (a n)"))
            return qT

        def A(h, qt, qT, yN):
            """one head x q-tile of windowed attention."""
            kvh = h // REP
            p = pwork.tile([128, NKC, 128], dt.bfloat16, tag="p")
            sa = saps.tile([128, NKA, 128], dt.float32, tag="sa")
            for kc in range(NKA):
                nc.tensor.matmul(sa[:, kc, :], lhsT=kT_sb[:, kvh, qt + kc, :],
                                 rhs=qT[:, h, :], start=True, stop=True)
            nc.scalar.activation(
                out=p[:, 0:NKA, :].rearrange("p a n -> p (a n)"),
                in_=sa.rearrange("p a n -> p (a n)"), func=AF.Exp)
            sb = sbps.tile([128, NKB, 128], dt.float32, tag="sb")
            for kc in range(NKB):
                nc.tensor.matmul(sb[:, kc, :],
                                 lhsT=kT_sb[:, kvh, qt + NKA + kc, :],
                                 rhs=qT[:, h, :], start=True, stop=True)
            nc.scalar.activation(
                out=p[:, NKA:NKC, :].rearrange("p a n -> p (a n)"),
                in_=sb.rearrange("p a n -> p (a n)"), func=AF.Exp)
            # triangular edge masks (left chunk kc=0, diag chunk kc=8)
            nc.gpsimd.tensor_mul(p[:, 0, :], p[:, 0, :], tri_sb[:, 0, :])
            nc.gpsimd.tensor_mul(p[:, NKC - 1, :], p[:, NKC - 1, :],
                                 tri_sb[:, 1, :])
            y = yps.tile([128, 129], dt.float32, tag="y")
            for kc in range(NKC):
                nc.tensor.matmul(y, lhsT=p[:, kc, :],
                                 rhs=v_sb[:, qt + kc, kvh, :],
                                 start=(kc == 0), stop=(kc == NKC - 1))
            # yN = y * (1/Z)  (Z = y[:,128], already pad-free via vmask col)
            z = work.tile([128, 1], dt.float32, tag="z")
            nc.vector.reciprocal(out=z, in_=y[:, 128:129])
            nc.vector.tensor_scalar_mul(yN[:, h, :], y[:, 0:128], z)

        def Dout(qt, yN):
            """transpose yN + output projection + store."""
            yT = pwork.tile([128, H, 128], dt.bfloat16, tag="yT")
            nc.sync.dma_start_transpose(out=yT,
                                        in_=yN.rearrange("p a n -> p (a n)"))
            for half in range(2):
                o = mmps.tile([128, 512], dt.float32, tag="mm")
                for h in range(H):
                    nc.tensor.matmul(o, lhsT=yT[:, h, :],
                                     rhs=wo_sb[:, h, bass.ts(half, 512)],
                                     start=(h == 0), stop=(h == H - 1))
                osb = work.tile([128, 512], dt.bfloat16, tag="osb")
                nc.vector.tensor_copy(out=osb, in_=o)
                nc.sync.dma_start(
                    out=out_d[bass.ts(qt, 128), bass.ts(half, 512)], in_=osb)

        # ---------------- emission schedule ----------------
        for rt in range(NQT + 1):      # K(0..8)
            K(rt)
        qT_cur = Q(0)
        yN_cur = pwork.tile([128, H, 128], dt.bfloat16, tag="yN")
        for qt in range(NQT):
            if qt + NQT + 1 < NRT:
                K(qt + NQT + 1)
            for h in range(H):
                A(h, qt, qT_cur, yN_cur)
                # prefetch next q tile early, between attention iterations
                if h == 1 and qt + 1 < NQT:
                    qT_nxt = Q(qt + 1)
            Dout(qt, yN_cur)
            if qt + 1 < NQT:
                qT_cur = qT_nxt
                yN_cur = pwork.tile([128, H, 128], dt.bfloat16, tag="yN")


# ---------------------------------------------------------------------------
# host side
# ---------------------------------------------------------------------------

def make_in_maps(x, ve, cos, sin, Wq, Wk, Wv, Wproj, Wg):
    """Build the 8 per-core input dicts (numpy, host-side prep)."""
    x = np.asarray(x, F32)
    ve = np.asarray(ve, F32)
    cos = np.asarray(cos, F32).reshape(T, 64)
    sin = np.asarray(sin, F32).reshape(T, 64)
    Wq = np.asarray(Wq, F32)
    Wk = np.asarray(Wk, F32)
    Wv = np.asarray(Wv, F32)
    Wproj = np.asarray(Wproj, F32)
    Wg = np.asarray(Wg, F32)

    wq = Wq.astype(BF16)
    wkv = np.concatenate([Wk, Wv], axis=1).astype(BF16)
    wo = Wproj.astype(BF16)
    wg = Wg.astype(BF16)

    # triangular masks in [k, q] layout
    kk = np.arange(128)[:, None]
    qq = np.arange(128)[None, :]
    tri = np.zeros((128, 2, 128), F32)
    tri[:, 0, :] = np.where(kk < qq, 0.0, 1.0)   # LEFT chunk (kc=0), mult mask
    tri[:, 1, :] = np.where(kk > qq, 0.0, 1.0)   # DIAG chunk (kc=8), mult mask
    tri = tri.reshape(128, 256).astype(BF16)

    in_maps = []
    for c in range(8):
        b, ck = divmod(c, 4)
        t0 = ck * RCHUNK
        es = t0 - WIN  # ext start (may be negative for chunk 0)
        pad = max(0, -es)

        def ext(a, fill_shape):
            out = np.zeros((E,) + fill_shape, F32)
            out[pad:] = a[es + pad: t0 + RCHUNK]
            return out

        x_e = ext(x[b], (C,))
        ve_e = ext(ve[b], (HKV * D,))
        cos_e = ext(cos, (64,))
        sin_e = ext(sin, (64,))

        # validity of each ext row, laid out [128, (rt, kvh)] to drop into
        # the augmented-V column: Z then automatically excludes padded rows.
        extpos = (np.arange(NRT)[None, :] * 128 + np.arange(128)[:, None])
        valid = (extpos >= pad).astype(F32)                  # [128, NRT]
        vmask = np.repeat(valid, HKV, axis=1).astype(BF16)   # [128, NRT*HKV]

        in_maps.append({
            "xT": np.ascontiguousarray(x_e.T).astype(BF16),
            "wq": wq, "wkv": wkv, "wo": wo, "wg": wg,
            "ve2": (2.0 * ve_e).astype(BF16),
            "cs": np.concatenate([cos_e, cos_e, -sin_e, sin_e],
                                 axis=1).astype(BF16),
            "tri": tri, "vmask": vmask,
        })
    return in_maps


_NC_CACHE = None


def kernel(x, ve, cos, sin, Wq, Wk, Wv, Wproj, Wg, window_size):
    assert int(window_size) == WIN
    global _NC_CACHE
    if _NC_CACHE is None:
        _NC_CACHE = build_nc()
    nc = _NC_CACHE
    in_maps = make_in_maps(x, ve, cos, sin, Wq, Wk, Wv, Wproj, Wg)
    res = bass_utils.run_bass_kernel_spmd(nc, in_maps, core_ids=list(range(8)))
    out = np.zeros((B, T, C), F32)
    for c in range(8):
        b, ck = divmod(c, 4)
        out[b, ck * RCHUNK:(ck + 1) * RCHUNK] = res.results[c]["out"].astype(F32)
    return out


# revision 23
# speedup vs baseline: 1.2173x; 1.0039x over previous
"""Sliding-window causal self-attention (GQA + RoPE + RMS-norm + value-embedding
gate) for Trainium2, sharded over 8 NeuronCores.

Sharding: sequence-parallel. (batch=2) x (4 sequence chunks of 1024) = 8 shards.
Each core computes attention for its own 1024 query rows. Window size = 1024 and
chunk size = 1024, so each core only needs K/V for its own chunk plus the
previous 1024 positions (halo). K/V (+rope/rms/gate) are recomputed locally for
the halo instead of communicated -> zero collectives. Chunk-0 shards get a
zero-padded halo; padded keys produce k=0 => exp(score)=1, which is excluded
from the softmax denominator by loading a per-row validity column into the
augmented-V "ones" column (padded v rows are 0 so the numerator is untouched).

v2 (fused pipeline):
- Single software-pipelined loop: kv-production K(rt), q-projection Q(qt),
  attention A(h,qt) and out-projection D(qt) interleave so the PE never idles
  between phases.
- All [128,128] transposes go through the DMA XBAR (dma_start_transpose)
  instead of PE matmul transposes + DVE psum->sbuf copies.
- Activation engine runs only {Exp, Square, Copy, Identity} (one table set,
  zero mid-kernel table reloads): the gate sigmoid is computed as
  1/(1+exp(-x)) and the RMS rsqrt as a DVE tensor_scalar pow(x, -0.5).
- RMS mean-square uses Act Square with fused accum_out.
- Rope multiplies/adds and edge masks on DVE (bf16 packed 4x mode); psum->sbuf
  copies and the v-gate fused multiply-add on the Pool engine.
- Scores are computed pre-transposed (s[k,q]) so exp's bf16 output is directly
  the lhsT of the PV matmul; V is augmented with the validity column so the PV
  matmul emits y[q,0:128] and the softmax denominator Z=y[q,128] in one
  accumulation group; normalization is a single DVE divide.
- Input DMAs are streamed in sequence slices so compute starts ~4us in.
"""

import math
import sys

import numpy as np

sys.path.insert(0, "/opt/trn_rl_repo")

import ml_dtypes

import concourse.bass as bass
import concourse.bacc as bacc
import concourse.tile as tile
from concourse import mybir
from concourse import bass_utils

BF16 = ml_dtypes.bfloat16
F32 = np.float32

B, T, C = 2, 4096, 1024
H, HKV, D = 8, 2, 128
REP = H // HKV
WIN = 1024
RCHUNK = 1024          # own rows per core
E = 2048               # ext rows (halo + own)
NRT = E // 128         # 16 ext row tiles
NQT = RCHUNK // 128    # 8 q tiles
NKC = 9                # k chunks per q tile
NKG = 3                # chunks per score/exp group (3 groups of 3)
NCT = C // 128         # 8 contraction tiles
NSL = 8                # xT/cs/ve DMA stream slices over E
SLE = E // NSL         # rows per stream slice
EPS = float(np.finfo(np.float32).eps)
SCALE = 1.0 / math.sqrt(D)

dt = mybir.dt
AF = mybir.ActivationFunctionType
ALU = mybir.AluOpType
AX = mybir.AxisListType


def _bcast(ap, n, axis_pos=1):
    """Insert a 0-stride dim of size n into an AP at free-axis position."""
    new_ap = list(ap.ap)
    new_ap.insert(axis_pos, [0, n])
    return bass.AP(tensor=ap.tensor, offset=ap.offset, ap=new_ap)


def _halfswap(ap, nh):
    """View [128, nh, 128] with the two 64-wide halves of the last dim
    swapped: out[p, h, 0:64] = in[p, h, 64:128] and vice versa."""
    base = list(ap.ap)
    return bass.AP(tensor=ap.tensor, offset=ap.offset + 64,
                   ap=[base[0], base[1], [-64, 2], [1, 64]])


def build_nc():
    nc = bacc.Bacc("TRN2", target_bir_lowering=False, debug=False)

    xT_d = nc.dram_tensor("xT", [C, E], dt.bfloat16, kind="ExternalInput").ap()
    wq_d = nc.dram_tensor("wq", [C, C], dt.bfloat16, kind="ExternalInput").ap()
    wkv_d = nc.dram_tensor("wkv", [C, 512], dt.bfloat16, kind="ExternalInput").ap()
    wo_d = nc.dram_tensor("wo", [C, C], dt.bfloat16, kind="ExternalInput").ap()
    wg_d = nc.dram_tensor("wg", [32, HKV], dt.bfloat16, kind="ExternalInput").ap()
    ve_d = nc.dram_tensor("ve2", [E, HKV * D], dt.bfloat16, kind="ExternalInput").ap()
    cs_d = nc.dram_tensor("cs", [E, 256], dt.bfloat16, kind="ExternalInput").ap()
    tri_d = nc.dram_tensor("tri", [128, 2 * 128], dt.bfloat16, kind="ExternalInput").ap()
    vm_d = nc.dram_tensor("vmask", [128, NRT * HKV], dt.bfloat16,
                          kind="ExternalInput").ap()
    out_d = nc.dram_tensor("out", [RCHUNK, C], dt.bfloat16, kind="ExternalOutput").ap()

    with tile.TileContext(nc) as tc:
        _body(tc, xT_d, wq_d, wkv_d, wo_d, wg_d, ve_d, cs_d, tri_d, vm_d, out_d)
    nc.compile()
    return nc


def _body(tc, xT_d, wq_d, wkv_d, wo_d, wg_d, ve_d, cs_d, tri_d, vm_d, out_d):
    nc = tc.nc
    from contextlib import ExitStack

    with ExitStack() as ctx:
        const = ctx.enter_context(tc.tile_pool(name="const", bufs=1))
        persist = ctx.enter_context(tc.tile_pool(name="persist", bufs=1))
        work = ctx.enter_context(tc.tile_pool(name="work", bufs=2))
        pwork = ctx.enter_context(tc.tile_pool(name="pwork", bufs=3))
        mmps = ctx.enter_context(tc.tile_pool(name="mmps", bufs=3, space="PSUM"))
        sps = ctx.enter_context(tc.tile_pool(name="sps", bufs=2, space="PSUM"))
        yps = ctx.enter_context(tc.tile_pool(name="yps", bufs=3, space="PSUM"))

        # ---- persistent SBUF ----
        wkv_sb = const.tile([128, NCT, 512], dt.bfloat16)
        wg_sb = const.tile([32, HKV], dt.bfloat16)
        xT_sb = const.tile([128, NCT, E], dt.bfloat16)
        cs_sb = const.tile([128, NRT, 256], dt.bfloat16)
        ve_sb = const.tile([128, NRT, HKV * D], dt.bfloat16)
        wq_sb = const.tile([128, NCT, C], dt.bfloat16)
        tri_sb = const.tile([128, 2, 128], dt.bfloat16)
        vm_sb = const.tile([128, NRT * HKV], dt.bfloat16)
        wo_sb = const.tile([128, NCT, C], dt.bfloat16)

        kT_sb = persist.tile([128, HKV, NRT, 128], dt.bfloat16)   # [d, kvh, g, k]
        v_sb = persist.tile([128, NRT, HKV, 129], dt.bfloat16)    # [k, g, kvh, d|m]
        gate_sb = persist.tile([128, NRT * HKV], dt.float32)      # [row, (g,kvh)]
        eg_sb = persist.tile([128, NRT * HKV], dt.float32)
        msk_sb = persist.tile([128, NRT, HKV], dt.float32)
        msq_sb = persist.tile([128, NQT, H], dt.float32)
        eps_sb = persist.tile([128, 1], dt.float32)
        lns_sb = persist.tile([128, 1], dt.float32)
        nc.vector.memset(eps_sb, EPS)
        nc.vector.memset(lns_sb, math.log(SCALE))

        # DMA views (partition-tiled DRAM)
        xTv = xT_d.rearrange("(a p) n -> p a n", p=128)
        csv = cs_d.rearrange("(a p) n -> p a n", p=128)
        vev = ve_d.rearrange("(a p) n -> p a n", p=128)

        # ---- streamed input DMAs, earliest-needed first ----
        nc.sync.dma_start(out=wkv_sb, in_=wkv_d.rearrange("(a p) n -> p a n", p=128))
        nc.sync.dma_start(out=wg_sb, in_=wg_d)

        def load_slice(sl):
            r = bass.ts(sl, SLE)
            nc.sync.dma_start(out=xT_sb[:, :, r], in_=xTv[:, :, r])
            rt0 = sl * (NRT // NSL)
            nc.sync.dma_start(out=cs_sb[:, rt0:rt0 + NRT // NSL, :],
                              in_=csv[:, rt0:rt0 + NRT // NSL, :])
            nc.sync.dma_start(out=ve_sb[:, rt0:rt0 + NRT // NSL, :],
                              in_=vev[:, rt0:rt0 + NRT // NSL, :])

        # xT one slice ahead of cs; ve (only needed by PV, not scores) trails
        # behind wq so the first attention group can start ~25us in.
        def load_xt(sl):
            r = bass.ts(sl, SLE)
            nc.sync.dma_start(out=xT_sb[:, :, r], in_=xTv[:, :, r])

        def load_aux(sl, which):
            rt0 = sl * (NRT // NSL)
            src = {"cs": (cs_sb, csv), "ve": (ve_sb, vev)}[which]
            nc.sync.dma_start(out=src[0][:, rt0:rt0 + NRT // NSL, :],
                              in_=src[1][:, rt0:rt0 + NRT // NSL, :])

        load_xt(0)
        load_xt(1)
        load_aux(0, "cs")
        load_xt(2)
        load_aux(1, "cs")
        load_xt(3)
        load_aux(2, "cs")
        load_xt(4)
        load_aux(3, "cs")
        load_aux(4, "cs")
        nc.sync.dma_start(out=wq_sb, in_=wq_d.rearrange("(a p) n -> p a n", p=128))
        nc.sync.dma_start(out=tri_sb, in_=tri_d.rearrange("p (a n) -> p a n", a=2))
        nc.sync.dma_start(out=vm_sb, in_=vm_d)
        for sl in range(5):
            load_aux(sl, "ve")
        for sl in range(5, NSL):
            load_xt(sl)
            load_aux(sl, "cs")
            load_aux(sl, "ve")
        nc.sync.dma_start(out=wo_sb, in_=wo_d.rearrange("(a p) n -> p a n", p=128))

        # Pin the act-function table to the one set containing every func we
        # use (Exp, Ln, Square, Copy, Identity) so the fixpoint table-load
        # pass never inserts another (otherwise Ln<->Exp thrashes 1.3us/load).
        from concourse.hw_specs import get_activation_tables
        _need = {AF.Exp, AF.Ln, AF.Square, AF.Copy, AF.Identity}
        _set_id = next(i for i, (_, s) in
                       enumerate(get_activation_tables(nc.m.arch).items())
                       if _need <= s)
        _lat = mybir.InstLoadActFuncSet(
            name=nc.get_next_instruction_name(), ins=[], outs=[],
            act_func_set_id=_set_id)
        _lat.engine = mybir.EngineType.Activation
        nc.scalar.add_instruction(_lat)

        # validity column of augmented V (kills the softmax pad correction)
        nc.vector.tensor_copy(
            out=v_sb[:, :, :, 128:129].rearrange("p a b n -> p (a b n)"),
            in_=vm_sb)

        # ---- gates: 2*sigmoid(u) = 2/(1+exp(-u)); the 2x is folded into ve2
        g_psum = mmps.tile([128, 512], dt.float32, tag="mm")
        for rt in range(NRT):
            nc.tensor.matmul(g_psum[:, bass.ts(rt, HKV)],
                             lhsT=xT_sb[0:32, 0, bass.ts(rt, 128)],
                             rhs=wg_sb, start=True, stop=True)
        nc.scalar.activation(out=eg_sb, in_=g_psum[:, 0:NRT * HKV],
                             func=AF.Exp, scale=-1.0)
        nc.vector.tensor_scalar_add(out=eg_sb, in0=eg_sb, scalar1=1.0)
        nc.vector.reciprocal(out=gate_sb, in_=eg_sb)

        # ---------------- pipeline stages ----------------
        def K(rt):
            """kv projection + gate-mix + rope + rms + normalize + kT."""
            rs = bass.ts(rt, 128)
            kv = mmps.tile([128, 512], dt.float32, tag="mm")
            for ct in range(NCT):
                nc.tensor.matmul(kv, lhsT=xT_sb[:, ct, rs], rhs=wkv_sb[:, ct, :],
                                 start=(ct == 0), stop=(ct == NCT - 1))
            # v = v_raw + gate*ve2
            for kvh in range(HKV):
                nc.vector.scalar_tensor_tensor(
                    out=v_sb[:, rt, kvh, 0:128],
                    in0=ve_sb[:, rt, bass.ts(kvh, 128)],
                    scalar=gate_sb[:, rt * HKV + kvh:rt * HKV + kvh + 1],
                    in1=kv[:, 256 + kvh * 128:256 + (kvh + 1) * 128],
                    op0=ALU.mult, op1=ALU.add)
            kraw = work.tile([128, HKV * D], dt.bfloat16, tag="kraw")
            nc.scalar.copy(out=kraw, in_=kv[:, 0:256])
            # rope (DVE, bf16 packed)
            k3 = kraw.rearrange("p (a n) -> p a n", a=HKV)
            kr = work.tile([128, HKV, D], dt.bfloat16, tag="krot")
            kc_ = work.tile([128, HKV, D], dt.bfloat16, tag="kc")
            ks_ = work.tile([128, HKV, D], dt.bfloat16, tag="ks")
            ccb = _bcast(cs_sb[:, rt, 0:128], HKV)
            ssb = _bcast(cs_sb[:, rt, 128:256], HKV)
            nc.vector.tensor_mul(kc_, k3, ccb)
            nc.vector.tensor_mul(ks_, k3, ssb)
            nc.vector.tensor_add(kr, kc_, _halfswap(ks_, HKV))
            # rms stats from the ROPED values (exact for any cos/sin)
            sqs = work.tile([128, HKV, D], dt.bfloat16, tag="sqs")
            nc.vector.tensor_mul(sqs, kr, kr)
            nc.vector.tensor_reduce(out=msk_sb[:, rt, :], in_=sqs,
                                    axis=AX.X, op=ALU.add)
            # c = rsqrt(ms/D + eps) = Exp(-0.5 * Ln(ms/D + eps)); Ln and Exp
            # live in one act table set so the Act engine never reloads.
            nc.scalar.activation(out=msk_sb[:, rt, :], in_=msk_sb[:, rt, :],
                                 func=AF.Ln, scale=1.0 / D, bias=eps_sb)
            nc.scalar.activation(out=msk_sb[:, rt, :], in_=msk_sb[:, rt, :],
                                 func=AF.Exp, scale=-0.5)
            for kvh in range(HKV):
                nc.vector.tensor_scalar_mul(
                    kr[:, kvh, :], kr[:, kvh, :], msk_sb[:, rt, kvh:kvh + 1])
            # one batched XBAR transpose for both kv heads: out[j,kvh,i]=in[i,kvh,j]
            nc.sync.dma_start_transpose(out=kT_sb[:, :, rt, :],
                                        in_=kr.rearrange("p a n -> p (a n)"))

        def Q(qt):
            """q projection + rope + rms + normalize(+scale) + qT."""
            rt = NQT + qt  # ext row tile of own rows
            rs = bass.ts(rt, 128)
            qraw = work.tile([128, C], dt.bfloat16, tag="qraw")
            for half in range(2):
                qp = mmps.tile([128, 512], dt.float32, tag="mm")
                for ct in range(NCT):
                    nc.tensor.matmul(qp, lhsT=xT_sb[:, ct, rs],
                                     rhs=wq_sb[:, ct, bass.ts(half, 512)],
                                     start=(ct == 0), stop=(ct == NCT - 1))
                nc.vector.tensor_copy(out=qraw[:, bass.ts(half, 512)], in_=qp)
            q3 = qraw.rearrange("p (a n) -> p a n", a=H)
            qr = work.tile([128, H, D], dt.bfloat16, tag="qrot")
            qc_ = work.tile([128, H, D], dt.bfloat16, tag="qc")
            qs_ = work.tile([128, H, D], dt.bfloat16, tag="qs")
            ccb = _bcast(cs_sb[:, rt, 0:128], H)
            ssb = _bcast(cs_sb[:, rt, 128:256], H)
            nc.vector.tensor_mul(qc_, q3, ccb)
            nc.vector.tensor_mul(qs_, q3, ssb)
            nc.vector.tensor_add(qr, qc_, _halfswap(qs_, H))
            sqs = work.tile([128, H, D], dt.bfloat16, tag="qsqs")
            nc.vector.tensor_mul(sqs, qr, qr)
            nc.vector.tensor_reduce(out=msq_sb[:, qt, :], in_=sqs,
                                    axis=AX.X, op=ALU.add)
            # c = rsqrt(ms/D + eps) * SCALE = Exp(-0.5*Ln(ms/D+eps) + ln SCALE)
            nc.scalar.activation(out=msq_sb[:, qt, :], in_=msq_sb[:, qt, :],
                                 func=AF.Ln, scale=1.0 / D, bias=eps_sb)
            nc.scalar.activation(out=msq_sb[:, qt, :], in_=msq_sb[:, qt, :],
                                 func=AF.Exp, scale=-0.5, bias=lns_sb)
            qT = pwork.tile([128, H, 128], dt.bfloat16, tag="qT")
            for h in range(H):
                nc.vector.tensor_scalar_mul(
                    qr[:, h, :], qr[:, h, :], msq_sb[:, qt, h:h + 1])
            nc.sync.dma_start_transpose(out=qT,
                                        in_=qr.rearrange("p a n -> p (a n)"))
            return qT

        def A(h, qt, qT, yN):
            """one head x q-tile of windowed attention."""
            kvh = h // REP
            p = pwork.tile([128, NKC, 128], dt.bfloat16, tag="p")
            for g in range(NKC // NKG):
                s = sps.tile([128, NKG, 128], dt.float32, tag="s")
                for j in range(NKG):
                    kc = g * NKG + j
                    nc.tensor.matmul(s[:, j, :],
                                     lhsT=kT_sb[:, kvh, qt + kc, :],
                                     rhs=qT[:, h, :], start=True, stop=True)
                nc.scalar.activation(
                    out=p[:, g * NKG:(g + 1) * NKG, :].rearrange(
                        "p a n -> p (a n)"),
                    in_=s.rearrange("p a n -> p (a n)"), func=AF.Exp)
                # triangular edge masks on the group just produced
                if g == 0:
                    nc.gpsimd.tensor_mul(p[:, 0, :], p[:, 0, :],
                                         tri_sb[:, 0, :])
                if g == NKC // NKG - 1:
                    nc.gpsimd.tensor_mul(p[:, NKC - 1, :], p[:, NKC - 1, :],
                                         tri_sb[:, 1, :])
            y = yps.tile([128, 129], dt.float32, tag="y")
            for kc in range(NKC):
                nc.tensor.matmul(y, lhsT=p[:, kc, :],
                                 rhs=v_sb[:, qt + kc, kvh, :],
                                 start=(kc == 0), stop=(kc == NKC - 1))
            # yN = y * (1/Z)  (Z = y[:,128], already pad-free via vmask col)
            z = work.tile([128, 1], dt.float32, tag="z")
            nc.vector.reciprocal(out=z, in_=y[:, 128:129])
            nc.vector.tensor_scalar_mul(yN[:, h, :], y[:, 0:128], z)

        def Dout(qt, yN):
            """transpose yN + output projection + store."""
            yT = pwork.tile([128, H, 128], dt.bfloat16, tag="yT")
            nc.sync.dma_start_transpose(out=yT,
                                        in_=yN.rearrange("p a n -> p (a n)"))
            for half in range(2):
                o = mmps.tile([128, 512], dt.float32, tag="mm")
                for h in range(H):
                    nc.tensor.matmul(o, lhsT=yT[:, h, :],
                                     rhs=wo_sb[:, h, bass.ts(half, 512)],
                                     start=(h == 0), stop=(h == H - 1))
                osb = work.tile([128, 512], dt.bfloat16, tag="osb")
                nc.vector.tensor_copy(out=osb, in_=o)
                nc.sync.dma_start(
                    out=out_d[bass.ts(qt, 128), bass.ts(half, 512)], in_=osb)

        # ---------------- emission schedule ----------------
        for rt in range(NQT + 1):      # K(0..8)
            K(rt)
        qT_cur = Q(0)
        yN_cur = pwork.tile([128, H, 128], dt.bfloat16, tag="yN")
        for qt in range(NQT):
            if qt + NQT + 1 < NRT:
                K(qt + NQT + 1)
            for h in range(H):
                A(h, qt, qT_cur, yN_cur)
                # prefetch next q tile early, between attention iterations
                if h == 1 and qt + 1 < NQT:
                    qT_nxt = Q(qt + 1)
            Dout(qt, yN_cur)
            if qt + 1 < NQT:
                qT_cur = qT_nxt
                yN_cur = pwork.tile([128, H, 128], dt.bfloat16, tag="yN")


# ---------------------------------------------------------------------------
# host side
# ---------------------------------------------------------------------------

def make_in_maps(x, ve, cos, sin, Wq, Wk, Wv, Wproj, Wg):
    """Build the 8 per-core input dicts (numpy, host-side prep)."""
    x = np.asarray(x, F32)
    ve = np.asarray(ve, F32)
    cos = np.asarray(cos, F32).reshape(T, 64)
    sin = np.asarray(sin, F32).reshape(T, 64)
    Wq = np.asarray(Wq, F32)
    Wk = np.asarray(Wk, F32)
    Wv = np.asarray(Wv, F32)
    Wproj = np.asarray(Wproj, F32)
    Wg = np.asarray(Wg, F32)

    wq = Wq.astype(BF16)
    wkv = np.concatenate([Wk, Wv], axis=1).astype(BF16)
    wo = Wproj.astype(BF16)
    wg = Wg.astype(BF16)

    # triangular masks in [k, q] layout
    kk = np.arange(128)[:, None]
    qq = np.arange(128)[None, :]
    tri = np.zeros((128, 2, 128), F32)
    tri[:, 0, :] = np.where(kk < qq, 0.0, 1.0)   # LEFT chunk (kc=0), mult mask
    tri[:, 1, :] = np.where(kk > qq, 0.0, 1.0)   # DIAG chunk (kc=8), mult mask
    tri = tri.reshape(128, 256).astype(BF16)

    in_maps = []
    for c in range(8):
        b, ck = divmod(c, 4)
        t0 = ck * RCHUNK
        es = t0 - WIN  # ext start (may be negative for chunk 0)
        pad = max(0, -es)

        def ext(a, fill_shape):
            out = np.zeros((E,) + fill_shape, F32)
            out[pad:] = a[es + pad: t0 + RCHUNK]
            return out

        x_e = ext(x[b], (C,))
        ve_e = ext(ve[b], (HKV * D,))
        cos_e = ext(cos, (64,))
        sin_e = ext(sin, (64,))

        # validity of each ext row, laid out [128, (rt, kvh)] to drop into
        # the augmented-V column: Z then automatically excludes padded rows.
        extpos = (np.arange(NRT)[None, :] * 128 + np.arange(128)[:, None])
        valid = (extpos >= pad).astype(F32)                  # [128, NRT]
        vmask = np.repeat(valid, HKV, axis=1).astype(BF16)   # [128, NRT*HKV]

        in_maps.append({
            "xT": np.ascontiguousarray(x_e.T).astype(BF16),
            "wq": wq, "wkv": wkv, "wo": wo, "wg": wg,
            "ve2": (2.0 * ve_e).astype(BF16),
            "cs": np.concatenate([cos_e, cos_e, -sin_e, sin_e],
                                 axis=1).astype(BF16),
            "tri": tri, "vmask": vmask,
        })
    return in_maps


_NC_CACHE = None


def kernel(x, ve, cos, sin, Wq, Wk, Wv, Wproj, Wg, window_size):
    assert int(window_size) == WIN
    global _NC_CACHE
    if _NC_CACHE is None:
        _NC_CACHE = build_nc()
    nc = _NC_CACHE
    in_maps = make_in_maps(x, ve, cos, sin, Wq, Wk, Wv, Wproj, Wg)
    res = bass_utils.run_bass_kernel_spmd(nc, in_maps, core_ids=list(range(8)))
    out = np.zeros((B, T, C), F32)
    for c in range(8):
        b, ck = divmod(c, 4)
        out[b, ck * RCHUNK:(ck + 1) * RCHUNK] = res.results[c]["out"].astype(F32)
    return out


# revision 31
# speedup vs baseline: 1.2577x; 1.0331x over previous
"""Sliding-window causal self-attention (GQA + RoPE + RMS-norm + value-embedding
gate) for Trainium2, sharded over 8 NeuronCores.

Sharding: sequence-parallel. (batch=2) x (4 sequence chunks of 1024) = 8 shards.
Each core computes attention for its own 1024 query rows. Window size = 1024 and
chunk size = 1024, so each core only needs K/V for its own chunk plus the
previous 1024 positions (halo). K/V (+rope/rms/gate) are recomputed locally for
the halo instead of communicated -> zero collectives. Chunk-0 shards get a
zero-padded halo; padded keys produce k=0 => exp(score)=1, which is excluded
from the softmax denominator by loading a per-row validity column into the
augmented-V "ones" column (padded v rows are 0 so the numerator is untouched).

v2 (fused pipeline):
- Single software-pipelined loop: kv-production K(rt), q-projection Q(qt),
  attention A(h,qt) and out-projection D(qt) interleave so the PE never idles
  between phases.
- All [128,128] transposes go through the DMA XBAR (dma_start_transpose)
  instead of PE matmul transposes + DVE psum->sbuf copies.
- Activation engine runs only {Exp, Square, Copy, Identity} (one table set,
  zero mid-kernel table reloads): the gate sigmoid is computed as
  1/(1+exp(-x)) and the RMS rsqrt as a DVE tensor_scalar pow(x, -0.5).
- RMS mean-square uses Act Square with fused accum_out.
- Rope multiplies/adds and edge masks on DVE (bf16 packed 4x mode); psum->sbuf
  copies and the v-gate fused multiply-add on the Pool engine.
- Scores are computed pre-transposed (s[k,q]) so exp's bf16 output is directly
  the lhsT of the PV matmul; V is augmented with the validity column so the PV
  matmul emits y[q,0:128] and the softmax denominator Z=y[q,128] in one
  accumulation group; normalization is a single DVE divide.
- Input DMAs are streamed in sequence slices so compute starts ~4us in.
"""

import math
import sys

import numpy as np

sys.path.insert(0, "/opt/trn_rl_repo")

import ml_dtypes

import concourse.bass as bass
import concourse.bacc as bacc
import concourse.tile as tile
from concourse import mybir
from concourse import bass_utils

BF16 = ml_dtypes.bfloat16
F32 = np.float32

B, T, C = 2, 4096, 1024
H, HKV, D = 8, 2, 128
REP = H // HKV
WIN = 1024
RCHUNK = 1024          # own rows per core
E = 2048               # ext rows (halo + own)
NRT = E // 128         # 16 ext row tiles
NQT = RCHUNK // 128    # 8 q tiles
NKC = 9                # k chunks per q tile
NKG = 3                # chunks per score/exp group (3 groups of 3)
NCT = C // 128         # 8 contraction tiles
NSL = 8                # xT/cs/ve DMA stream slices over E
SLE = E // NSL         # rows per stream slice
EPS = float(np.finfo(np.float32).eps)
SCALE = 1.0 / math.sqrt(D)

dt = mybir.dt
AF = mybir.ActivationFunctionType
ALU = mybir.AluOpType
AX = mybir.AxisListType


def _bcast(ap, n, axis_pos=1):
    """Insert a 0-stride dim of size n into an AP at free-axis position."""
    new_ap = list(ap.ap)
    new_ap.insert(axis_pos, [0, n])
    return bass.AP(tensor=ap.tensor, offset=ap.offset, ap=new_ap)


def _halfswap(ap, nh):
    """View [128, nh, 128] with the two 64-wide halves of the last dim
    swapped: out[p, h, 0:64] = in[p, h, 64:128] and vice versa."""
    base = list(ap.ap)
    return bass.AP(tensor=ap.tensor, offset=ap.offset + 64,
                   ap=[base[0], base[1], [-64, 2], [1, 64]])


def build_nc():
    nc = bacc.Bacc("TRN2", target_bir_lowering=False, debug=False)

    xT_d = nc.dram_tensor("xT", [C, E], dt.bfloat16, kind="ExternalInput").ap()
    wq_d = nc.dram_tensor("wq", [C, C], dt.bfloat16, kind="ExternalInput").ap()
    wkv_d = nc.dram_tensor("wkv", [C, 512], dt.bfloat16, kind="ExternalInput").ap()
    wo_d = nc.dram_tensor("wo", [C, C], dt.bfloat16, kind="ExternalInput").ap()
    wg_d = nc.dram_tensor("wg", [32, HKV], dt.bfloat16, kind="ExternalInput").ap()
    ve_d = nc.dram_tensor("ve2", [E, HKV * D], dt.bfloat16, kind="ExternalInput").ap()
    cs_d = nc.dram_tensor("cs", [E, 256], dt.bfloat16, kind="ExternalInput").ap()
    tri_d = nc.dram_tensor("tri", [128, 2 * 128], dt.bfloat16, kind="ExternalInput").ap()
    vm_d = nc.dram_tensor("vmask", [128, NRT * HKV], dt.bfloat16,
                          kind="ExternalInput").ap()
    out_d = nc.dram_tensor("out", [RCHUNK, C], dt.bfloat16, kind="ExternalOutput").ap()

    with tile.TileContext(nc) as tc:
        _body(tc, xT_d, wq_d, wkv_d, wo_d, wg_d, ve_d, cs_d, tri_d, vm_d, out_d)
    nc.compile()
    return nc


def _body(tc, xT_d, wq_d, wkv_d, wo_d, wg_d, ve_d, cs_d, tri_d, vm_d, out_d):
    nc = tc.nc
    from contextlib import ExitStack

    with ExitStack() as ctx:
        const = ctx.enter_context(tc.tile_pool(name="const", bufs=1))
        persist = ctx.enter_context(tc.tile_pool(name="persist", bufs=1))
        work = ctx.enter_context(tc.tile_pool(name="work", bufs=2))
        pwork = ctx.enter_context(tc.tile_pool(name="pwork", bufs=3))
        mmps = ctx.enter_context(tc.tile_pool(name="mmps", bufs=3, space="PSUM"))
        sps = ctx.enter_context(tc.tile_pool(name="sps", bufs=2, space="PSUM"))
        yps = ctx.enter_context(tc.tile_pool(name="yps", bufs=2, space="PSUM"))
        gps = ctx.enter_context(tc.tile_pool(name="gps", bufs=1, space="PSUM"))

        # ---- persistent SBUF ----
        wkv_sb = const.tile([128, NCT, 512], dt.bfloat16)
        wg_sb = const.tile([32, HKV], dt.bfloat16)
        xT_sb = const.tile([128, NCT, E], dt.bfloat16)
        cs_sb = const.tile([128, NRT, 256], dt.bfloat16)
        ve_sb = const.tile([128, NRT, HKV * D], dt.bfloat16)
        wq_sb = const.tile([128, NCT, C], dt.bfloat16)
        tri_sb = const.tile([128, 2, 128], dt.bfloat16)
        vm_sb = const.tile([128, NRT * HKV], dt.bfloat16)
        wo_sb = const.tile([128, NCT, C], dt.bfloat16)

        kT_sb = persist.tile([128, HKV, NRT, 128], dt.bfloat16)   # [d, kvh, g, k]
        v_sb = persist.tile([128, NRT, HKV, 129], dt.bfloat16)    # [k, g, kvh, d|m]
        gate_sb = persist.tile([128, NRT * HKV], dt.float32)      # [row, (g,kvh)]
        eg_sb = persist.tile([128, NRT * HKV], dt.float32)
        msk_sb = persist.tile([128, NRT, HKV], dt.float32)
        msq_sb = persist.tile([128, NQT, H], dt.float32)
        eps_sb = persist.tile([128, 1], dt.float32)
        lns_sb = persist.tile([128, 1], dt.float32)
        nc.vector.memset(eps_sb, EPS)
        nc.vector.memset(lns_sb, math.log(SCALE))

        # DMA views (partition-tiled DRAM)
        xTv = xT_d.rearrange("(a p) n -> p a n", p=128)
        csv = cs_d.rearrange("(a p) n -> p a n", p=128)
        vev = ve_d.rearrange("(a p) n -> p a n", p=128)

        # ---- streamed input DMAs, earliest-needed first ----
        nc.sync.dma_start(out=wkv_sb, in_=wkv_d.rearrange("(a p) n -> p a n", p=128))
        nc.sync.dma_start(out=wg_sb, in_=wg_d)

        def load_slice(sl):
            r = bass.ts(sl, SLE)
            nc.sync.dma_start(out=xT_sb[:, :, r], in_=xTv[:, :, r])
            rt0 = sl * (NRT // NSL)
            nc.sync.dma_start(out=cs_sb[:, rt0:rt0 + NRT // NSL, :],
                              in_=csv[:, rt0:rt0 + NRT // NSL, :])
            nc.sync.dma_start(out=ve_sb[:, rt0:rt0 + NRT // NSL, :],
                              in_=vev[:, rt0:rt0 + NRT // NSL, :])

        # xT one slice ahead of cs; ve (only needed by PV, not scores) trails
        # behind wq so the first attention group can start ~25us in.
        wqv = wq_d.rearrange("(a p) n -> p a n", p=128)

        def load_xt(sl):
            r = bass.ts(sl, SLE)
            nc.sync.dma_start(out=xT_sb[:, :, r], in_=xTv[:, :, r])

        def load_aux(sl, which):
            rt0 = sl * (NRT // NSL)
            src = {"cs": (cs_sb, csv), "ve": (ve_sb, vev)}[which]
            nc.sync.dma_start(out=src[0][:, rt0:rt0 + NRT // NSL, :],
                              in_=src[1][:, rt0:rt0 + NRT // NSL, :])

        load_xt(0)
        load_xt(1)
        load_aux(0, "cs")
        load_xt(2)
        load_aux(1, "cs")
        load_xt(3)
        load_aux(2, "cs")
        load_xt(4)
        load_aux(3, "cs")
        load_aux(4, "cs")
        nc.sync.dma_start(out=tri_sb, in_=tri_d.rearrange("p (a n) -> p a n", a=2))
        nc.sync.dma_start(out=vm_sb, in_=vm_d)
        nc.sync.dma_start(out=wq_sb[:, :, 0:512], in_=wqv[:, :, 0:512])
        nc.sync.dma_start(out=wq_sb[:, :, 512:1024], in_=wqv[:, :, 512:1024])
        for sl in range(5):
            load_aux(sl, "ve")
        for sl in range(5, NSL):
            load_xt(sl)
            load_aux(sl, "cs")
            load_aux(sl, "ve")
        nc.sync.dma_start(out=wo_sb, in_=wo_d.rearrange("(a p) n -> p a n", p=128))

        # Pin the act-function table to the one set containing every func we
        # use (Exp, Ln, Square, Copy, Identity) so the fixpoint table-load
        # pass never inserts another (otherwise Ln<->Exp thrashes 1.3us/load).
        from concourse.hw_specs import get_activation_tables
        _need = {AF.Exp, AF.Ln, AF.Square, AF.Copy, AF.Identity}
        _set_id = next(i for i, (_, s) in
                       enumerate(get_activation_tables(nc.m.arch).items())
                       if _need <= s)
        _lat = mybir.InstLoadActFuncSet(
            name=nc.get_next_instruction_name(), ins=[], outs=[],
            act_func_set_id=_set_id)
        _lat.engine = mybir.EngineType.Activation
        nc.scalar.add_instruction(_lat)

        # validity column of augmented V (kills the softmax pad correction)
        nc.vector.tensor_copy(
            out=v_sb[:, :, :, 128:129].rearrange("p a b n -> p (a b n)"),
            in_=vm_sb)

        # ---- gates: 2*sigmoid(u) = 2/(1+exp(-u)); the 2x is folded into ve2.
        # Processed per xT-slice so early v-tiles don't wait on late slices.
        g_psum = gps.tile([128, NRT * HKV], dt.float32)

        def gates(sl):
            r0 = sl * (NRT // NSL)
            for rt in range(r0, r0 + NRT // NSL):
                nc.tensor.matmul(g_psum[:, bass.ts(rt, HKV)],
                                 lhsT=xT_sb[0:32, 0, bass.ts(rt, 128)],
                                 rhs=wg_sb, start=True, stop=True)
            cols = bass.ts(sl, NRT // NSL * HKV)
            nc.scalar.activation(out=eg_sb[:, cols], in_=g_psum[:, cols],
                                 func=AF.Exp, scale=-1.0)
            nc.vector.tensor_scalar_add(out=eg_sb[:, cols], in0=eg_sb[:, cols],
                                        scalar1=1.0)
            nc.vector.reciprocal(out=gate_sb[:, cols], in_=eg_sb[:, cols])

        # ---------------- pipeline stages ----------------
        def K_mm(rt, lo, hi, kv):
            rs = bass.ts(rt, 128)
            for ct in range(lo, hi):
                nc.tensor.matmul(kv, lhsT=xT_sb[:, ct, rs], rhs=wkv_sb[:, ct, :],
                                 start=(ct == 0), stop=(ct == NCT - 1))

        def K_post(rt, kv):
            """gate-mix + rope + rms + normalize + kT from the kv psum."""
            # v = v_raw + gate*ve2
            for kvh in range(HKV):
                nc.vector.scalar_tensor_tensor(
                    out=v_sb[:, rt, kvh, 0:128],
                    in0=ve_sb[:, rt, bass.ts(kvh, 128)],
                    scalar=gate_sb[:, rt * HKV + kvh:rt * HKV + kvh + 1],
                    in1=kv[:, 256 + kvh * 128:256 + (kvh + 1) * 128],
                    op0=ALU.mult, op1=ALU.add)
            kraw = work.tile([128, HKV * D], dt.bfloat16, tag="kraw")
            nc.scalar.copy(out=kraw, in_=kv[:, 0:256])
            # rope (DVE, bf16 packed)
            k3 = kraw.rearrange("p (a n) -> p a n", a=HKV)
            kr = work.tile([128, HKV, D], dt.bfloat16, tag="krot")
            kc_ = work.tile([128, HKV, D], dt.bfloat16, tag="kc")
            ks_ = work.tile([128, HKV, D], dt.bfloat16, tag="ks")
            ccb = _bcast(cs_sb[:, rt, 0:128], HKV)
            ssb = _bcast(cs_sb[:, rt, 128:256], HKV)
            nc.vector.tensor_mul(kc_, k3, ccb)
            nc.vector.tensor_mul(ks_, k3, ssb)
            nc.vector.tensor_add(kr, kc_, _halfswap(ks_, HKV))
            # rms stats from the ROPED values (exact for any cos/sin)
            sqs = work.tile([128, HKV, D], dt.bfloat16, tag="sqs")
            nc.vector.tensor_mul(sqs, kr, kr)
            nc.vector.tensor_reduce(out=msk_sb[:, rt, :], in_=sqs,
                                    axis=AX.X, op=ALU.add)
            # c = rsqrt(ms/D + eps) = Exp(-0.5 * Ln(ms/D + eps)); Ln and Exp
            # live in one act table set so the Act engine never reloads.
            nc.scalar.activation(out=msk_sb[:, rt, :], in_=msk_sb[:, rt, :],
                                 func=AF.Ln, scale=1.0 / D, bias=eps_sb)
            nc.scalar.activation(out=msk_sb[:, rt, :], in_=msk_sb[:, rt, :],
                                 func=AF.Exp, scale=-0.5)
            for kvh in range(HKV):
                nc.vector.tensor_scalar_mul(
                    kr[:, kvh, :], kr[:, kvh, :], msk_sb[:, rt, kvh:kvh + 1])
            # one batched XBAR transpose for both kv heads: out[j,kvh,i]=in[i,kvh,j]
            nc.sync.dma_start_transpose(out=kT_sb[:, :, rt, :],
                                        in_=kr.rearrange("p a n -> p (a n)"))

        def K(rt):
            kv = mmps.tile([128, 512], dt.float32, tag="mm")
            K_mm(rt, 0, NCT, kv)
            K_post(rt, kv)

        HH = H // 2  # heads per wq half

        def Q_mm(qt, half, qp):
            rs = bass.ts(NQT + qt, 128)
            for ct in range(NCT):
                nc.tensor.matmul(qp, lhsT=xT_sb[:, ct, rs],
                                 rhs=wq_sb[:, ct, bass.ts(half, 512)],
                                 start=(ct == 0), stop=(ct == NCT - 1))

        def Q_post(qt, half, qp, qT):
            """rope + rms + normalize(+scale) + qT for one 4-head half."""
            rt = NQT + qt
            qraw = work.tile([128, HH, D], dt.bfloat16, tag="qraw")
            nc.vector.tensor_copy(out=qraw.rearrange("p a n -> p (a n)"),
                                  in_=qp)
            qr = work.tile([128, HH, D], dt.bfloat16, tag="qrot")
            qc_ = work.tile([128, HH, D], dt.bfloat16, tag="qc")
            qs_ = work.tile([128, HH, D], dt.bfloat16, tag="qs")
            ccb = _bcast(cs_sb[:, rt, 0:128], HH)
            ssb = _bcast(cs_sb[:, rt, 128:256], HH)
            nc.vector.tensor_mul(qc_, qraw, ccb)
            nc.vector.tensor_mul(qs_, qraw, ssb)
            nc.vector.tensor_add(qr, qc_, _halfswap(qs_, HH))
            sqs = work.tile([128, HH, D], dt.bfloat16, tag="qsqs")
            nc.vector.tensor_mul(sqs, qr, qr)
            ms = msq_sb[:, qt, half * HH:(half + 1) * HH]
            nc.vector.tensor_reduce(out=ms, in_=sqs, axis=AX.X, op=ALU.add)
            # c = rsqrt(ms/D + eps) * SCALE = Exp(-0.5*Ln(ms/D+eps) + ln SCALE)
            nc.scalar.activation(out=ms, in_=ms, func=AF.Ln, scale=1.0 / D,
                                 bias=eps_sb)
            nc.scalar.activation(out=ms, in_=ms, func=AF.Exp, scale=-0.5,
                                 bias=lns_sb)
            for j in range(HH):
                nc.vector.tensor_scalar_mul(
                    qr[:, j, :], qr[:, j, :], ms[:, j:j + 1])
            nc.sync.dma_start_transpose(
                out=qT[:, half * HH:(half + 1) * HH, :],
                in_=qr.rearrange("p a n -> p (a n)"))

        def Q_half(qt, half, qT):
            qp = mmps.tile([128, 512], dt.float32, tag="mm")
            Q_mm(qt, half, qp)
            Q_post(qt, half, qp, qT)

        def A(h, qt, qT, yN):
            """one head x q-tile of windowed attention."""
            kvh = h // REP
            p = pwork.tile([128, NKC, 128], dt.bfloat16, tag="p")
            for g in range(NKC // NKG):
                s = sps.tile([128, NKG, 128], dt.float32, tag="s")
                for j in range(NKG):
                    kc = g * NKG + j
                    nc.tensor.matmul(s[:, j, :],
                                     lhsT=kT_sb[:, kvh, qt + kc, :],
                                     rhs=qT[:, h, :], start=True, stop=True)
                nc.scalar.activation(
                    out=p[:, g * NKG:(g + 1) * NKG, :].rearrange(
                        "p a n -> p (a n)"),
                    in_=s.rearrange("p a n -> p (a n)"), func=AF.Exp)
                # triangular edge masks on the group just produced
                if g == 0:
                    nc.gpsimd.tensor_mul(p[:, 0, :], p[:, 0, :],
                                         tri_sb[:, 0, :])
                if g == NKC // NKG - 1:
                    nc.gpsimd.tensor_mul(p[:, NKC - 1, :], p[:, NKC - 1, :],
                                         tri_sb[:, 1, :])
            y = yps.tile([128, 129], dt.float32, tag="y")
            for kc in range(NKC):
                nc.tensor.matmul(y, lhsT=p[:, kc, :],
                                 rhs=v_sb[:, qt + kc, kvh, :],
                                 start=(kc == 0), stop=(kc == NKC - 1))
            # yN = y * (1/Z)  (Z = y[:,128], already pad-free via vmask col)
            z = work.tile([128, 1], dt.float32, tag="z")
            nc.vector.reciprocal(out=z, in_=y[:, 128:129])
            nc.vector.tensor_scalar_mul(yN[:, h, :], y[:, 0:128], z)

        def D_tp(yN):
            """XBAR-transpose the full yN tile (all 8 heads, one DMA)."""
            yT = pwork.tile([128, H, 128], dt.bfloat16, tag="yT")
            nc.sync.dma_start_transpose(out=yT,
                                        in_=yN.rearrange("p a n -> p (a n)"))
            return yT

        def D_half(qt, half, yT):
            o = mmps.tile([128, 512], dt.float32, tag="mm")
            for h in range(H):
                nc.tensor.matmul(o, lhsT=yT[:, h, :],
                                 rhs=wo_sb[:, h, bass.ts(half, 512)],
                                 start=(h == 0), stop=(h == H - 1))
            osb = work.tile([128, 512], dt.bfloat16, tag="osb")
            nc.vector.tensor_copy(out=osb, in_=o)
            nc.sync.dma_start(
                out=out_d[bass.ts(qt, 128), bass.ts(half, 512)], in_=osb)

        # ---------------- emission schedule ----------------
        # Prologue: gates for the first 5 slices, kv for the 9 tiles the
        # first q tile attends to, then q tile 0 (per wq half).
        for rt in range(NQT + 1):      # K(0..8), gates interleaved per slice
            if rt % 2 == 0:
                gates(rt // 2)
            K(rt)
        qT_cur = pwork.tile([128, H, 128], dt.bfloat16, tag="qT")
        Q_half(0, 0, qT_cur)
        Q_half(0, 1, qT_cur)
        yN_cur = pwork.tile([128, H, 128], dt.bfloat16, tag="yN")
        yT_prev = None

        # Steady state: each qt window runs 8 attention iterations with the
        # independent PE work (out-proj of qt-1, kv of qt+9, q of qt+1)
        # spread between them so the PE queue never head-of-line blocks on
        # the scores->exp chain.
        for qt in range(NQT):
            krt = qt + NQT + 1
            kv = None
            for h in range(H):
                A(h, qt, qT_cur, yN_cur)
                if h == 0 and yT_prev is not None:
                    D_half(qt - 1, 0, yT_prev)
                if h == 1 and yT_prev is not None:
                    D_half(qt - 1, 1, yT_prev)
                if h == 2 and krt < NRT:
                    if krt % 2 == 0 and krt // 2 >= 5:
                        gates(krt // 2)  # gates for this slice pair
                    kv = mmps.tile([128, 512], dt.float32, tag="mm")
                    K_mm(krt, 0, NCT // 2, kv)
                if h == 3 and kv is not None:
                    K_mm(krt, NCT // 2, NCT, kv)
                    K_post(krt, kv)
                if h == 4 and qt + 1 < NQT:
                    qT_nxt = pwork.tile([128, H, 128], dt.bfloat16, tag="qT")
                    Q_half(qt + 1, 0, qT_nxt)
                if h == 6 and qt + 1 < NQT:
                    Q_half(qt + 1, 1, qT_nxt)
            yT_prev = D_tp(yN_cur)
            if qt + 1 < NQT:
                qT_cur = qT_nxt
                yN_cur = pwork.tile([128, H, 128], dt.bfloat16, tag="yN")
        D_half(NQT - 1, 0, yT_prev)
        D_half(NQT - 1, 1, yT_prev)


# ---------------------------------------------------------------------------
# host side
# ---------------------------------------------------------------------------

def make_in_maps(x, ve, cos, sin, Wq, Wk, Wv, Wproj, Wg):
    """Build the 8 per-core input dicts (numpy, host-side prep)."""
    x = np.asarray(x, F32)
    ve = np.asarray(ve, F32)
    cos = np.asarray(cos, F32).reshape(T, 64)
    sin = np.asarray(sin, F32).reshape(T, 64)
    Wq = np.asarray(Wq, F32)
    Wk = np.asarray(Wk, F32)
    Wv = np.asarray(Wv, F32)
    Wproj = np.asarray(Wproj, F32)
    Wg = np.asarray(Wg, F32)

    wq = Wq.astype(BF16)
    wkv = np.concatenate([Wk, Wv], axis=1).astype(BF16)
    wo = Wproj.astype(BF16)
    wg = Wg.astype(BF16)

    # triangular masks in [k, q] layout
    kk = np.arange(128)[:, None]
    qq = np.arange(128)[None, :]
    tri = np.zeros((128, 2, 128), F32)
    tri[:, 0, :] = np.where(kk < qq, 0.0, 1.0)   # LEFT chunk (kc=0), mult mask
    tri[:, 1, :] = np.where(kk > qq, 0.0, 1.0)   # DIAG chunk (kc=8), mult mask
    tri = tri.reshape(128, 256).astype(BF16)

    in_maps = []
    for c in range(8):
        b, ck = divmod(c, 4)
        t0 = ck * RCHUNK
        es = t0 - WIN  # ext start (may be negative for chunk 0)
        pad = max(0, -es)

        def ext(a, fill_shape):
            out = np.zeros((E,) + fill_shape, F32)
            out[pad:] = a[es + pad: t0 + RCHUNK]
            return out

        x_e = ext(x[b], (C,))
        ve_e = ext(ve[b], (HKV * D,))
        cos_e = ext(cos, (64,))
        sin_e = ext(sin, (64,))

        # validity of each ext row, laid out [128, (rt, kvh)] to drop into
        # the augmented-V column: Z then automatically excludes padded rows.
        extpos = (np.arange(NRT)[None, :] * 128 + np.arange(128)[:, None])
        valid = (extpos >= pad).astype(F32)                  # [128, NRT]
        vmask = np.repeat(valid, HKV, axis=1).astype(BF16)   # [128, NRT*HKV]

        in_maps.append({
            "xT": np.ascontiguousarray(x_e.T).astype(BF16),
            "wq": wq, "wkv": wkv, "wo": wo, "wg": wg,
            "ve2": (2.0 * ve_e).astype(BF16),
            "cs": np.concatenate([cos_e, cos_e, -sin_e, sin_e],
                                 axis=1).astype(BF16),
            "tri": tri, "vmask": vmask,
        })
    return in_maps


_NC_CACHE = None


def kernel(x, ve, cos, sin, Wq, Wk, Wv, Wproj, Wg, window_size):
    assert int(window_size) == WIN
    global _NC_CACHE
    if _NC_CACHE is None:
        _NC_CACHE = build_nc()
    nc = _NC_CACHE
    in_maps = make_in_maps(x, ve, cos, sin, Wq, Wk, Wv, Wproj, Wg)
    res = bass_utils.run_bass_kernel_spmd(nc, in_maps, core_ids=list(range(8)))
    out = np.zeros((B, T, C), F32)
    for c in range(8):
        b, ck = divmod(c, 4)
        out[b, ck * RCHUNK:(ck + 1) * RCHUNK] = res.results[c]["out"].astype(F32)
    return out


# revision 33
# speedup vs baseline: 1.3781x; 1.0958x over previous
"""Sliding-window causal self-attention (GQA + RoPE + RMS-norm + value-embedding
gate) for Trainium2, sharded over 8 NeuronCores.

Sharding: sequence-parallel. (batch=2) x (4 sequence chunks of 1024) = 8 shards.
Each core computes attention for its own 1024 query rows. Window size = 1024 and
chunk size = 1024, so each core only needs K/V for its own chunk plus the
previous 1024 positions (halo). K/V (+rope/rms/gate) are recomputed locally for
the halo instead of communicated -> zero collectives. Chunk-0 shards get a
zero-padded halo; padded keys produce k=0 => exp(score)=1, which is excluded
from the softmax denominator by loading a per-row validity column into the
augmented-V "ones" column (padded v rows are 0 so the numerator is untouched).

v2 (fused pipeline):
- Single software-pipelined loop: kv-production K(rt), q-projection Q(qt),
  attention A(h,qt) and out-projection D(qt) interleave so the PE never idles
  between phases.
- All [128,128] transposes go through the DMA XBAR (dma_start_transpose)
  instead of PE matmul transposes + DVE psum->sbuf copies.
- Activation engine runs only {Exp, Square, Copy, Identity} (one table set,
  zero mid-kernel table reloads): the gate sigmoid is computed as
  1/(1+exp(-x)) and the RMS rsqrt as a DVE tensor_scalar pow(x, -0.5).
- RMS mean-square uses Act Square with fused accum_out.
- Rope multiplies/adds and edge masks on DVE (bf16 packed 4x mode); psum->sbuf
  copies and the v-gate fused multiply-add on the Pool engine.
- Scores are computed pre-transposed (s[k,q]) so exp's bf16 output is directly
  the lhsT of the PV matmul; V is augmented with the validity column so the PV
  matmul emits y[q,0:128] and the softmax denominator Z=y[q,128] in one
  accumulation group; normalization is a single DVE divide.
- Input DMAs are streamed in sequence slices so compute starts ~4us in.
"""

import math
import sys

import numpy as np

sys.path.insert(0, "/opt/trn_rl_repo")

import ml_dtypes

import concourse.bass as bass
import concourse.bacc as bacc
import concourse.tile as tile
from concourse import mybir
from concourse import bass_utils

BF16 = ml_dtypes.bfloat16
F32 = np.float32

B, T, C = 2, 4096, 1024
H, HKV, D = 8, 2, 128
REP = H // HKV
WIN = 1024
RCHUNK = 1024          # own rows per core
E = 2048               # ext rows (halo + own)
NRT = E // 128         # 16 ext row tiles
NQT = RCHUNK // 128    # 8 q tiles
NKC = 9                # k chunks per q tile
NKG = 3                # chunks per score/exp group (3 groups of 3)
NCT = C // 128         # 8 contraction tiles
NSL = 8                # xT/cs/ve DMA stream slices over E
SLE = E // NSL         # rows per stream slice
EPS = float(np.finfo(np.float32).eps)
SCALE = 1.0 / math.sqrt(D)

dt = mybir.dt
AF = mybir.ActivationFunctionType
ALU = mybir.AluOpType
AX = mybir.AxisListType


def _bcast(ap, n, axis_pos=1):
    """Insert a 0-stride dim of size n into an AP at free-axis position."""
    new_ap = list(ap.ap)
    new_ap.insert(axis_pos, [0, n])
    return bass.AP(tensor=ap.tensor, offset=ap.offset, ap=new_ap)


def _halfswap(ap, nh):
    """View [128, nh, 128] with the two 64-wide halves of the last dim
    swapped: out[p, h, 0:64] = in[p, h, 64:128] and vice versa."""
    base = list(ap.ap)
    return bass.AP(tensor=ap.tensor, offset=ap.offset + 64,
                   ap=[base[0], base[1], [-64, 2], [1, 64]])


def build_nc():
    nc = bacc.Bacc("TRN2", target_bir_lowering=False, debug=False)

    xT_d = nc.dram_tensor("xT", [C, E], dt.bfloat16, kind="ExternalInput").ap()
    wq_d = nc.dram_tensor("wq", [C, C], dt.bfloat16, kind="ExternalInput").ap()
    wkv_d = nc.dram_tensor("wkv", [C, 512], dt.bfloat16, kind="ExternalInput").ap()
    wo_d = nc.dram_tensor("wo", [C, C], dt.bfloat16, kind="ExternalInput").ap()
    wg_d = nc.dram_tensor("wg", [32, HKV], dt.bfloat16, kind="ExternalInput").ap()
    ve_d = nc.dram_tensor("ve2", [E, HKV * D], dt.bfloat16, kind="ExternalInput").ap()
    cs_d = nc.dram_tensor("cs", [E, 256], dt.bfloat16, kind="ExternalInput").ap()
    tri_d = nc.dram_tensor("tri", [128, 2 * 128], dt.bfloat16, kind="ExternalInput").ap()
    vm_d = nc.dram_tensor("vmask", [128, NRT * HKV], dt.bfloat16,
                          kind="ExternalInput").ap()
    out_d = nc.dram_tensor("out", [RCHUNK, C], dt.bfloat16, kind="ExternalOutput").ap()

    with tile.TileContext(nc) as tc:
        _body(tc, xT_d, wq_d, wkv_d, wo_d, wg_d, ve_d, cs_d, tri_d, vm_d, out_d)
    nc.compile()
    return nc


def _body(tc, xT_d, wq_d, wkv_d, wo_d, wg_d, ve_d, cs_d, tri_d, vm_d, out_d):
    nc = tc.nc
    from contextlib import ExitStack

    with ExitStack() as ctx:
        const = ctx.enter_context(tc.tile_pool(name="const", bufs=1))
        persist = ctx.enter_context(tc.tile_pool(name="persist", bufs=1))
        work = ctx.enter_context(tc.tile_pool(name="work", bufs=2))
        pwork = ctx.enter_context(tc.tile_pool(name="pwork", bufs=3))
        mmps = ctx.enter_context(tc.tile_pool(name="mmps", bufs=3, space="PSUM"))
        sps = ctx.enter_context(tc.tile_pool(name="sps", bufs=2, space="PSUM"))
        yps = ctx.enter_context(tc.tile_pool(name="yps", bufs=2, space="PSUM"))
        gps = ctx.enter_context(tc.tile_pool(name="gps", bufs=1, space="PSUM"))

        # ---- persistent SBUF ----
        wkv_sb = const.tile([128, NCT, 512], dt.bfloat16)
        wg_sb = const.tile([32, HKV], dt.bfloat16)
        xT_sb = const.tile([128, NCT, E], dt.bfloat16)
        cs_sb = const.tile([128, NRT, 256], dt.bfloat16)
        ve_sb = const.tile([128, NRT, HKV * D], dt.bfloat16)
        wq_sb = const.tile([128, NCT, C], dt.bfloat16)
        tri_sb = const.tile([128, 2, 128], dt.bfloat16)
        vm_sb = const.tile([128, NRT * HKV], dt.bfloat16)
        wo_sb = const.tile([128, NCT, C], dt.bfloat16)

        kT_sb = persist.tile([128, HKV, NRT, 128], dt.bfloat16)   # [d, kvh, g, k]
        v_sb = persist.tile([128, NRT, HKV, 129], dt.bfloat16)    # [k, g, kvh, d|m]
        gate_sb = persist.tile([128, NRT * HKV], dt.float32)      # [row, (g,kvh)]
        eg_sb = persist.tile([128, NRT * HKV], dt.float32)
        msk_sb = persist.tile([128, NRT, HKV], dt.float32)
        msq_sb = persist.tile([128, NQT, H], dt.float32)
        eps_sb = persist.tile([128, 1], dt.float32)
        lns_sb = persist.tile([128, 1], dt.float32)
        nc.vector.memset(eps_sb, EPS)
        nc.vector.memset(lns_sb, math.log(SCALE))

        # DMA views (partition-tiled DRAM)
        xTv = xT_d.rearrange("(a p) n -> p a n", p=128)
        csv = cs_d.rearrange("(a p) n -> p a n", p=128)
        vev = ve_d.rearrange("(a p) n -> p a n", p=128)

        # ---- streamed input DMAs, earliest-needed first ----
        nc.sync.dma_start(out=wkv_sb, in_=wkv_d.rearrange("(a p) n -> p a n", p=128))
        nc.sync.dma_start(out=wg_sb, in_=wg_d)

        def load_slice(sl):
            r = bass.ts(sl, SLE)
            nc.sync.dma_start(out=xT_sb[:, :, r], in_=xTv[:, :, r])
            rt0 = sl * (NRT // NSL)
            nc.sync.dma_start(out=cs_sb[:, rt0:rt0 + NRT // NSL, :],
                              in_=csv[:, rt0:rt0 + NRT // NSL, :])
            nc.sync.dma_start(out=ve_sb[:, rt0:rt0 + NRT // NSL, :],
                              in_=vev[:, rt0:rt0 + NRT // NSL, :])

        # xT one slice ahead of cs; ve (only needed by PV, not scores) trails
        # behind wq so the first attention group can start ~25us in.
        wqv = wq_d.rearrange("(a p) n -> p a n", p=128)

        def load_xt(sl):
            r = bass.ts(sl, SLE)
            nc.sync.dma_start(out=xT_sb[:, :, r], in_=xTv[:, :, r])

        def load_aux(sl, which):
            rt0 = sl * (NRT // NSL)
            src = {"cs": (cs_sb, csv), "ve": (ve_sb, vev)}[which]
            nc.sync.dma_start(out=src[0][:, rt0:rt0 + NRT // NSL, :],
                              in_=src[1][:, rt0:rt0 + NRT // NSL, :])

        load_xt(0)
        load_xt(1)
        load_aux(0, "cs")
        load_xt(2)
        load_aux(1, "cs")
        load_xt(3)
        load_aux(2, "cs")
        load_xt(4)
        load_aux(3, "cs")
        load_aux(4, "cs")
        nc.sync.dma_start(out=tri_sb, in_=tri_d.rearrange("p (a n) -> p a n", a=2))
        nc.sync.dma_start(out=vm_sb, in_=vm_d)
        nc.sync.dma_start(out=wq_sb[:, :, 0:512], in_=wqv[:, :, 0:512])
        nc.sync.dma_start(out=wq_sb[:, :, 512:1024], in_=wqv[:, :, 512:1024])
        for sl in range(5):
            load_aux(sl, "ve")
        for sl in range(5, NSL):
            load_xt(sl)
            load_aux(sl, "cs")
            load_aux(sl, "ve")
        nc.sync.dma_start(out=wo_sb, in_=wo_d.rearrange("(a p) n -> p a n", p=128))

        # Pin the act-function table to the one set containing every func we
        # use (Exp, Ln, Square, Copy, Identity) so the fixpoint table-load
        # pass never inserts another (otherwise Ln<->Exp thrashes 1.3us/load).
        from concourse.hw_specs import get_activation_tables
        _need = {AF.Exp, AF.Ln, AF.Square, AF.Copy, AF.Identity}
        _set_id = next(i for i, (_, s) in
                       enumerate(get_activation_tables(nc.m.arch).items())
                       if _need <= s)
        _lat = mybir.InstLoadActFuncSet(
            name=nc.get_next_instruction_name(), ins=[], outs=[],
            act_func_set_id=_set_id)
        _lat.engine = mybir.EngineType.Activation
        nc.scalar.add_instruction(_lat)

        # validity column of augmented V (kills the softmax pad correction)
        nc.vector.tensor_copy(
            out=v_sb[:, :, :, 128:129].rearrange("p a b n -> p (a b n)"),
            in_=vm_sb)

        # ---- gates: 2*sigmoid(u) = 2/(1+exp(-u)); the 2x is folded into ve2.
        # Processed per xT-slice so early v-tiles don't wait on late slices.
        g_psum = gps.tile([128, NRT * HKV], dt.float32)

        def gates(sl):
            r0 = sl * (NRT // NSL)
            for rt in range(r0, r0 + NRT // NSL):
                nc.tensor.matmul(g_psum[:, bass.ts(rt, HKV)],
                                 lhsT=xT_sb[0:32, 0, bass.ts(rt, 128)],
                                 rhs=wg_sb, start=True, stop=True)
            cols = bass.ts(sl, NRT // NSL * HKV)
            nc.scalar.activation(out=eg_sb[:, cols], in_=g_psum[:, cols],
                                 func=AF.Exp, scale=-1.0)
            nc.vector.tensor_scalar_add(out=eg_sb[:, cols], in0=eg_sb[:, cols],
                                        scalar1=1.0)
            nc.vector.reciprocal(out=gate_sb[:, cols], in_=eg_sb[:, cols])

        # ---------------- pipeline stages ----------------
        def K_mm(rt, lo, hi, kv):
            rs = bass.ts(rt, 128)
            for ct in range(lo, hi):
                nc.tensor.matmul(kv, lhsT=xT_sb[:, ct, rs], rhs=wkv_sb[:, ct, :],
                                 start=(ct == 0), stop=(ct == NCT - 1))

        def K_post(rt, kv):
            """gate-mix + rope + rms + normalize + kT from the kv psum."""
            # v = v_raw + gate*ve2
            for kvh in range(HKV):
                nc.vector.scalar_tensor_tensor(
                    out=v_sb[:, rt, kvh, 0:128],
                    in0=ve_sb[:, rt, bass.ts(kvh, 128)],
                    scalar=gate_sb[:, rt * HKV + kvh:rt * HKV + kvh + 1],
                    in1=kv[:, 256 + kvh * 128:256 + (kvh + 1) * 128],
                    op0=ALU.mult, op1=ALU.add)
            kraw = work.tile([128, HKV * D], dt.bfloat16, tag="kraw")
            nc.scalar.copy(out=kraw, in_=kv[:, 0:256])
            # rope (DVE, bf16 packed)
            k3 = kraw.rearrange("p (a n) -> p a n", a=HKV)
            kr = work.tile([128, HKV, D], dt.bfloat16, tag="krot")
            kc_ = work.tile([128, HKV, D], dt.bfloat16, tag="kc")
            ks_ = work.tile([128, HKV, D], dt.bfloat16, tag="ks")
            ccb = _bcast(cs_sb[:, rt, 0:128], HKV)
            ssb = _bcast(cs_sb[:, rt, 128:256], HKV)
            nc.vector.tensor_mul(kc_, k3, ccb)
            nc.vector.tensor_mul(ks_, k3, ssb)
            nc.vector.tensor_add(kr, kc_, _halfswap(ks_, HKV))
            # rms stats from the ROPED values (exact for any cos/sin)
            sqs = work.tile([128, HKV, D], dt.bfloat16, tag="sqs")
            nc.vector.tensor_mul(sqs, kr, kr)
            nc.vector.tensor_reduce(out=msk_sb[:, rt, :], in_=sqs,
                                    axis=AX.X, op=ALU.add)
            # c = rsqrt(ms/D + eps) = Exp(-0.5 * Ln(ms/D + eps)); Ln and Exp
            # live in one act table set so the Act engine never reloads.
            nc.scalar.activation(out=msk_sb[:, rt, :], in_=msk_sb[:, rt, :],
                                 func=AF.Ln, scale=1.0 / D, bias=eps_sb)
            nc.scalar.activation(out=msk_sb[:, rt, :], in_=msk_sb[:, rt, :],
                                 func=AF.Exp, scale=-0.5)
            for kvh in range(HKV):
                nc.vector.tensor_scalar_mul(
                    kr[:, kvh, :], kr[:, kvh, :], msk_sb[:, rt, kvh:kvh + 1])
            # one batched XBAR transpose for both kv heads: out[j,kvh,i]=in[i,kvh,j]
            nc.sync.dma_start_transpose(out=kT_sb[:, :, rt, :],
                                        in_=kr.rearrange("p a n -> p (a n)"))

        def K(rt):
            kv = mmps.tile([128, 512], dt.float32, tag="mm")
            K_mm(rt, 0, NCT, kv)
            K_post(rt, kv)

        HH = H // 2  # heads per wq half

        def Q_mm(qt, half, qp):
            rs = bass.ts(NQT + qt, 128)
            for ct in range(NCT):
                nc.tensor.matmul(qp, lhsT=xT_sb[:, ct, rs],
                                 rhs=wq_sb[:, ct, bass.ts(half, 512)],
                                 start=(ct == 0), stop=(ct == NCT - 1))

        def Q_post(qt, half, qp, qT):
            """rope + rms + normalize(+scale) + qT for one 4-head half."""
            rt = NQT + qt
            qraw = work.tile([128, HH, D], dt.bfloat16, tag="qraw")
            nc.vector.tensor_copy(out=qraw.rearrange("p a n -> p (a n)"),
                                  in_=qp)
            qr = work.tile([128, HH, D], dt.bfloat16, tag="qrot")
            qc_ = work.tile([128, HH, D], dt.bfloat16, tag="qc")
            qs_ = work.tile([128, HH, D], dt.bfloat16, tag="qs")
            ccb = _bcast(cs_sb[:, rt, 0:128], HH)
            ssb = _bcast(cs_sb[:, rt, 128:256], HH)
            nc.vector.tensor_mul(qc_, qraw, ccb)
            nc.vector.tensor_mul(qs_, qraw, ssb)
            nc.vector.tensor_add(qr, qc_, _halfswap(qs_, HH))
            sqs = work.tile([128, HH, D], dt.bfloat16, tag="qsqs")
            nc.vector.tensor_mul(sqs, qr, qr)
            ms = msq_sb[:, qt, half * HH:(half + 1) * HH]
            nc.vector.tensor_reduce(out=ms, in_=sqs, axis=AX.X, op=ALU.add)
            # c = rsqrt(ms/D + eps) * SCALE = Exp(-0.5*Ln(ms/D+eps) + ln SCALE)
            nc.scalar.activation(out=ms, in_=ms, func=AF.Ln, scale=1.0 / D,
                                 bias=eps_sb)
            nc.scalar.activation(out=ms, in_=ms, func=AF.Exp, scale=-0.5,
                                 bias=lns_sb)
            for j in range(HH):
                nc.vector.tensor_scalar_mul(
                    qr[:, j, :], qr[:, j, :], ms[:, j:j + 1])
            nc.sync.dma_start_transpose(
                out=qT[:, half * HH:(half + 1) * HH, :],
                in_=qr.rearrange("p a n -> p (a n)"))

        def Q_half(qt, half, qT):
            qp = mmps.tile([128, 512], dt.float32, tag="mm")
            Q_mm(qt, half, qp)
            Q_post(qt, half, qp, qT)

        def A_scores(h, qt, qT):
            """scores + exp + edge masks for one head x q-tile."""
            kvh = h // REP
            p = pwork.tile([128, NKC, 128], dt.bfloat16, tag="p")
            for g in range(NKC // NKG):
                s = sps.tile([128, NKG, 128], dt.float32, tag="s")
                for j in range(NKG):
                    kc = g * NKG + j
                    nc.tensor.matmul(s[:, j, :],
                                     lhsT=kT_sb[:, kvh, qt + kc, :],
                                     rhs=qT[:, h, :], start=True, stop=True)
                nc.scalar.activation(
                    out=p[:, g * NKG:(g + 1) * NKG, :].rearrange(
                        "p a n -> p (a n)"),
                    in_=s.rearrange("p a n -> p (a n)"), func=AF.Exp)
                # triangular edge masks on the group just produced
                if g == 0:
                    nc.gpsimd.tensor_mul(p[:, 0, :], p[:, 0, :],
                                         tri_sb[:, 0, :])
                if g == NKC // NKG - 1:
                    nc.gpsimd.tensor_mul(p[:, NKC - 1, :], p[:, NKC - 1, :],
                                         tri_sb[:, 1, :])
            return p

        def A_pv(h, qt, p, yN):
            kvh = h // REP
            y = yps.tile([128, 129], dt.float32, tag="y")
            for kc in range(NKC):
                nc.tensor.matmul(y, lhsT=p[:, kc, :],
                                 rhs=v_sb[:, qt + kc, kvh, :],
                                 start=(kc == 0), stop=(kc == NKC - 1))
            # yN = y * (1/Z)  (Z = y[:,128], already pad-free via vmask col)
            z = work.tile([128, 1], dt.float32, tag="z")
            nc.vector.reciprocal(out=z, in_=y[:, 128:129])
            nc.vector.tensor_scalar_mul(yN[:, h, :], y[:, 0:128], z)

        def D_tp(yN):
            """XBAR-transpose the full yN tile (all 8 heads, one DMA)."""
            yT = pwork.tile([128, H, 128], dt.bfloat16, tag="yT")
            nc.sync.dma_start_transpose(out=yT,
                                        in_=yN.rearrange("p a n -> p (a n)"))
            return yT

        def D_half(qt, half, yT):
            o = mmps.tile([128, 512], dt.float32, tag="mm")
            for h in range(H):
                nc.tensor.matmul(o, lhsT=yT[:, h, :],
                                 rhs=wo_sb[:, h, bass.ts(half, 512)],
                                 start=(h == 0), stop=(h == H - 1))
            osb = work.tile([128, 512], dt.bfloat16, tag="osb")
            nc.vector.tensor_copy(out=osb, in_=o)
            nc.sync.dma_start(
                out=out_d[bass.ts(qt, 128), bass.ts(half, 512)], in_=osb)

        # ---------------- emission schedule ----------------
        # Prologue: gates for the first 5 slices, kv for the 9 tiles the
        # first q tile attends to, then q tile 0 (per wq half).
        for rt in range(NQT + 1):      # K(0..8), gates interleaved per slice
            if rt % 2 == 0:
                gates(rt // 2)
            K(rt)
        qT_cur = pwork.tile([128, H, 128], dt.bfloat16, tag="qT")
        Q_half(0, 0, qT_cur)
        Q_half(0, 1, qT_cur)
        yN_cur = pwork.tile([128, H, 128], dt.bfloat16, tag="yN")
        yT_prev = None

        # Steady state, software-pipelined one iteration deep: the PE queue
        # per iteration is [scores(i) | pv(i-1) | side-work], so the exp(i)
        # latency is hidden behind pv(i-1) and the interleaved independent
        # matmuls (out-proj of qt-1, q proj of qt+1, kv of qt+9) instead of
        # head-of-line blocking the engine.
        pend = None           # (h, qt, p, yN) whose PV is not yet emitted
        yT_prev, dprev_qt = None, None
        for qt in range(NQT):
            krt = qt + NQT + 1
            kv = None
            for h in range(H):
                p = A_scores(h, qt, qT_cur)
                if pend is not None:
                    A_pv(*pend)
                    if pend[0] == H - 1:  # last pv of previous window
                        yT_prev, dprev_qt = D_tp(pend[3]), pend[1]
                pend = (h, qt, p, yN_cur)
                if h == 2 and qt + 1 < NQT:
                    qT_nxt = pwork.tile([128, H, 128], dt.bfloat16, tag="qT")
                    Q_half(qt + 1, 0, qT_nxt)
                if h == 3 and yT_prev is not None:
                    D_half(dprev_qt, 0, yT_prev)
                if h == 4 and qt + 1 < NQT:
                    Q_half(qt + 1, 1, qT_nxt)
                if h == 5 and yT_prev is not None:
                    D_half(dprev_qt, 1, yT_prev)
                if h == 6 and krt < NRT:
                    if krt % 2 == 0 and krt // 2 >= 5:
                        gates(krt // 2)  # gates for this slice pair
                    kv = mmps.tile([128, 512], dt.float32, tag="mm")
                    K_mm(krt, 0, NCT // 2, kv)
                if h == 7 and kv is not None:
                    K_mm(krt, NCT // 2, NCT, kv)
                    K_post(krt, kv)
            if qt + 1 < NQT:
                qT_cur = qT_nxt
                yN_cur = pwork.tile([128, H, 128], dt.bfloat16, tag="yN")
        A_pv(*pend)
        yT_prev = D_tp(pend[3])
        D_half(NQT - 1, 0, yT_prev)
        D_half(NQT - 1, 1, yT_prev)


# ---------------------------------------------------------------------------
# host side
# ---------------------------------------------------------------------------

def make_in_maps(x, ve, cos, sin, Wq, Wk, Wv, Wproj, Wg):
    """Build the 8 per-core input dicts (numpy, host-side prep)."""
    x = np.asarray(x, F32)
    ve = np.asarray(ve, F32)
    cos = np.asarray(cos, F32).reshape(T, 64)
    sin = np.asarray(sin, F32).reshape(T, 64)
    Wq = np.asarray(Wq, F32)
    Wk = np.asarray(Wk, F32)
    Wv = np.asarray(Wv, F32)
    Wproj = np.asarray(Wproj, F32)
    Wg = np.asarray(Wg, F32)

    wq = Wq.astype(BF16)
    wkv = np.concatenate([Wk, Wv], axis=1).astype(BF16)
    wo = Wproj.astype(BF16)
    wg = Wg.astype(BF16)

    # triangular masks in [k, q] layout
    kk = np.arange(128)[:, None]
    qq = np.arange(128)[None, :]
    tri = np.zeros((128, 2, 128), F32)
    tri[:, 0, :] = np.where(kk < qq, 0.0, 1.0)   # LEFT chunk (kc=0), mult mask
    tri[:, 1, :] = np.where(kk > qq, 0.0, 1.0)   # DIAG chunk (kc=8), mult mask
    tri = tri.reshape(128, 256).astype(BF16)

    in_maps = []
    for c in range(8):
        b, ck = divmod(c, 4)
        t0 = ck * RCHUNK
        es = t0 - WIN  # ext start (may be negative for chunk 0)
        pad = max(0, -es)

        def ext(a, fill_shape):
            out = np.zeros((E,) + fill_shape, F32)
            out[pad:] = a[es + pad: t0 + RCHUNK]
            return out

        x_e = ext(x[b], (C,))
        ve_e = ext(ve[b], (HKV * D,))
        cos_e = ext(cos, (64,))
        sin_e = ext(sin, (64,))

        # validity of each ext row, laid out [128, (rt, kvh)] to drop into
        # the augmented-V column: Z then automatically excludes padded rows.
        extpos = (np.arange(NRT)[None, :] * 128 + np.arange(128)[:, None])
        valid = (extpos >= pad).astype(F32)                  # [128, NRT]
        vmask = np.repeat(valid, HKV, axis=1).astype(BF16)   # [128, NRT*HKV]

        in_maps.append({
            "xT": np.ascontiguousarray(x_e.T).astype(BF16),
            "wq": wq, "wkv": wkv, "wo": wo, "wg": wg,
            "ve2": (2.0 * ve_e).astype(BF16),
            "cs": np.concatenate([cos_e, cos_e, -sin_e, sin_e],
                                 axis=1).astype(BF16),
            "tri": tri, "vmask": vmask,
        })
    return in_maps


_NC_CACHE = None


def kernel(x, ve, cos, sin, Wq, Wk, Wv, Wproj, Wg, window_size):
    assert int(window_size) == WIN
    global _NC_CACHE
    if _NC_CACHE is None:
        _NC_CACHE = build_nc()
    nc = _NC_CACHE
    in_maps = make_in_maps(x, ve, cos, sin, Wq, Wk, Wv, Wproj, Wg)
    res = bass_utils.run_bass_kernel_spmd(nc, in_maps, core_ids=list(range(8)))
    out = np.zeros((B, T, C), F32)
    for c in range(8):
        b, ck = divmod(c, 4)
        out[b, ck * RCHUNK:(ck + 1) * RCHUNK] = res.results[c]["out"].astype(F32)
    return out
